# revision 7
# baseline (speedup 1.0000x reference)
"""Trainium2 Bass kernel v2 for nn_LCAMatrixModel (pairwise selu-MLP scoring).

o[i,j] = hardsigmoid( sum_h W2b[h]*selu(g[i,h]+g[j,h]+b2a[h]) + b2b ), symmetric.

Circulant band decomposition: each core owns 192 consecutive global rows
(core c: rows [192c, 192c+192)) of the band t[i, d] = o[i, (i+d) % N],
d in [0, 769) - exactly half the matrix. Inputs are host-rolled by 192c so
all cores run the same program on local rows [0, 192). Rows are processed
as 96 pairs (i, i+96) stacked on 128 partitions (2 x 64 h).

Per-pair math over the band window (u = g_i + g_j + b2a):
  sum_h w*selu(u) = lam*al*sum_h w*(rs + q) - lam*al*Sw,  with
  rs = relu(u)/al   and   q = e^{min(u,0)} = min(e^{g_i} * e^{g_j+b2a}, 1)
(the exp is SEPARABLE: E=e^g per node is precomputed, so the pairwise q is
just a multiply+min). o = clip(V/6 + 0.5, 0, 1). Every matmul uses the one
stationary pattern P1 = lam*al*w (g is stored pre-divided by al).

Engine balance via per-pair schemes (slot counts tunable):
  Z  : rs = DVE ts(add,max0);  q = DVE ts(mult,min1);  PE: P1@q + P1@rs
  Y2 : rs = ACT Relu(g2w+bias); q = DVE;               PE: P1@q + P1@rs
  W2 : rs = ACT; q = DVE; f = q+rs (DVE or Pool tt);   PE: P1@f  (half PE)
"""
import sys

sys.path.insert(0, "/opt/trn_rl_repo")

import numpy as np

N_NODES = 1536
RAW = 512
D = 128
H = 64
NCORES = 8
ROWS = N_NODES // NCORES          # 192 rows per core
PAIRS = ROWS // 2                 # 96 pairs (i, i+96)
GROUPS = 3                        # 3 groups of 32 pairs
BW = 772                          # padded band width (769 used)
GSPAN = 960                       # g2w width: max i (95) + BW + pad
XCOLS = 1056                      # g cols needed: GSPAN + 96 (bottom shift)
CW0, CW1 = 356, BW - 356          # psum chunk widths (356+416)
ECW = 352                         # encoder/prologue chunk width (3 x 352 = 1056)

LAM = 1.0507009873554805
AL = 1.6732632423543772

# per-group scheme quotas (fractions of 32): Z=DVE rs, Y2=ACT rs + 2mm,
# W2=ACT rs + f-combine + 1mm. Of W2, NWP_FR on Pool, rest DVE.
NZ_DEF = 15
NY_DEF = 3
NWP_DEF = 7
GROUP_SIZES = (32, 32, 32)
# per-group (NZ, NY, NW, NWP): front-load PE work, drain PE-light
GROUP_QUOTAS = None  # uniform per-group quotas

_compiled = None


def _build_program(NZ=NZ_DEF, NY=NY_DEF, NWP=NWP_DEF):
    import concourse.bacc as bacc
    import concourse.mybir as mybir
    import concourse.tile as tile

    F32 = mybir.dt.float32
    BF16 = mybir.dt.bfloat16
    AF = mybir.ActivationFunctionType
    OP = mybir.AluOpType

    NW = 32 - NZ - NY

    nc = bacc.Bacc("TRN2", target_bir_lowering=False, debug=False)

    # ---- DRAM I/O ----
    xT_d = nc.dram_tensor("xT", [D, 4 * XCOLS], BF16, kind="ExternalInput")
    cpack_d = nc.dram_tensor("cpack", [D, 5], F32, kind="ExternalInput")
    wab_d = nc.dram_tensor("wab", [D, 6 * H], BF16, kind="ExternalInput")
    w1bT_d = nc.dram_tensor("w1bT", [H, D], BF16, kind="ExternalInput")
    p1_d = nc.dram_tensor("p1", [D, 32 * H], BF16, kind="ExternalInput")
    out_d = nc.dram_tensor("out", [ROWS, BW], BF16, kind="ExternalOutput")

    with tile.TileContext(nc) as tc:
        with (
            tc.tile_pool(name="cst", bufs=1) as cst,
            tc.tile_pool(name="enc", bufs=1) as enc,
            tc.tile_pool(name="pre", bufs=3) as pre,
            tc.tile_pool(name="fq", bufs=28) as fq,
            tc.tile_pool(name="op", bufs=6) as opool,
            tc.tile_pool(name="ps", bufs=2, space="PSUM") as ps,
            tc.tile_pool(name="psp", bufs=4, space="PSUM") as psp,
        ):
            # ---- input DMAs (x first - encoder needs it asap) ----
            xt = enc.tile([D, 4 * XCOLS], BF16)
            nc.scalar.dma_start(xt[:, :], xT_d[:, :])
            cpack = cst.tile([D, 5], F32)
            nc.sync.dma_start(cpack[:], cpack_d[:])
            wab = cst.tile([D, 6 * H], BF16)
            nc.sync.dma_start(wab[:], wab_d[:])
            w1bT = cst.tile([H, D], BF16)
            nc.sync.dma_start(w1bT[:, :], w1bT_d[:])
            p1 = cst.tile([D, 32 * H], BF16)
            nc.sync.dma_start(p1[:], p1_d[:])
            b1b = cpack[:, 1:2]
            b2a2 = cpack[:, 2:3]
            bcmb = cpack[0:H, 3:4]
            kcol = cpack[0:H, 4:5]
            wcmb = wab[:, 0 : 4 * H]       # W1a@W_enc, 4 k-tiles of [128,64]
            w2aT = wab[:, 4 * H : 6 * H]
            NCH = XCOLS // ECW  # 3 chunks of 352

            # ---- selu helper: v = psum+b (bf16), then 4x-mode DVE ops ----
            def selu_from_psum(out_ap, pa, b_raw, p, nm):
                v = pre.tile([p, ECW], BF16, tag="selv", name=f"v_{nm}")
                nc.vector.tensor_scalar(v[:], pa, b_raw, 0.0, OP.add, OP.add)
                r = pre.tile([p, ECW], BF16, tag="selr", name=f"r_{nm}")
                nc.vector.tensor_scalar(r[:], v[:], 0.0, LAM, OP.max, OP.mult)
                m = pre.tile([p, ECW], BF16, tag="selm", name=f"m_{nm}")
                nc.vector.tensor_scalar_min(m[:], v[:], 0.0)
                e = pre.tile([p, ECW], BF16, tag="sele", name=f"e_{nm}")
                nc.scalar.activation(e[:], m[:], AF.Exp)
                t = pre.tile([p, ECW], BF16, tag="selt", name=f"t_{nm}")
                nc.vector.tensor_scalar(t[:], e[:], LAM * AL, -LAM * AL, OP.mult, OP.add)
                nc.vector.tensor_tensor(out_ap, r[:], t[:], OP.add)

            # ---- wavefront emission over (layer, chunk): keeps every
            # engine queue supplied while chains pipeline ----
            a1T = enc.tile([H, XCOLS], BF16)
            hT = enc.tile([D, XCOLS], BF16)
            g2dup = cst.tile([D, XCOLS], F32)
            g2w = cst.tile([D, GSPAN], BF16)
            E2w = cst.tile([D, GSPAN], BF16)
            gbs_raw = cst.tile([D, PAIRS], F32)
            gbs_s = cst.tile([D, PAIRS], F32)
            EIS = cst.tile([D, PAIRS], F32)

            def do_a1(c):
                # fused encoder+first layer: a1pre = x @ (W1a@W_enc).T
                sl = slice(c * ECW, (c + 1) * ECW)
                pa = psp.tile([H, ECW], F32, tag="pp", name=f"pa_{c}")
                for k in range(4):
                    nc.tensor.matmul(
                        pa[:],
                        wcmb[:, k * H : (k + 1) * H],
                        xt[:, k * XCOLS + c * ECW : k * XCOLS + (c + 1) * ECW],
                        start=(k == 0),
                        stop=(k == 3),
                    )
                selu_from_psum(a1T[:, sl], pa[:], bcmb, H, f"a{c}")

            def do_h(c):
                sl = slice(c * ECW, (c + 1) * ECW)
                ph = psp.tile([D, ECW], F32, tag="pp", name=f"ph_{c}")
                nc.tensor.matmul(ph[:], w1bT[:, :], a1T[:, sl], start=True, stop=True)
                selu_from_psum(hT[:, sl], ph[:], b1b, D, f"h{c}")

            def do_g(c):
                sl = slice(c * ECW, (c + 1) * ECW)
                pg = psp.tile([D, ECW], F32, tag="pp", name=f"pg_{c}")
                nc.tensor.matmul(pg[:], w2aT, hT[:, sl], start=True, stop=True)
                nc.scalar.activation(g2dup[:, sl], pg[:], AF.Copy)
                # build the g2w/E2w pieces this chunk enables (top: same cols,
                # bottom: cols shifted by -96), so the main loop can start
                # right after the last chunk
                t0, t1 = c * ECW, min((c + 1) * ECW, GSPAN)
                if t1 > t0:
                    nc.gpsimd.tensor_scalar_mul(
                        g2w[0:H, t0:t1], g2dup[0:H, t0:t1], 1.0 / AL
                    )
                    nc.scalar.activation(
                        E2w[0:H, t0:t1], g2dup[0:H, t0:t1], AF.Exp,
                        bias=b2a2[0:H, :],
                    )
                b0 = max(c * ECW - 96, 0)
                b1 = min((c + 1) * ECW - 96, GSPAN)
                if b1 > b0:
                    nc.gpsimd.tensor_scalar_mul(
                        g2w[H:D, b0:b1], g2dup[H:D, 96 + b0 : 96 + b1], 1.0 / AL
                    )
                    nc.scalar.activation(
                        E2w[H:D, b0:b1], g2dup[H:D, 96 + b0 : 96 + b1], AF.Exp,
                        bias=b2a2[H:D, :],
                    )
                if c == 0:
                    nc.gpsimd.tensor_copy(gbs_raw[0:H, :], g2dup[0:H, 0:PAIRS])
                    nc.gpsimd.tensor_copy(
                        gbs_raw[H:D, :], g2dup[H:D, 96 : 96 + PAIRS]
                    )
                    nc.vector.tensor_scalar(
                        gbs_s[:], gbs_raw[:], b2a2, 1.0 / AL, OP.add, OP.mult
                    )
                    nc.scalar.activation(EIS[:], gbs_raw[:], AF.Exp)

            layers = (do_a1, do_h, do_g)
            for step in range(len(layers) + NCH - 1):
                for c in range(NCH):
                    li = step - c
                    if 0 <= li < len(layers):
                        layers[li](c)

            # ================= main pairwise loop =================
            def finalize(G, pb, gsz, pt, c0, cw):
                # o = Relu(psum/6 + kb) ; out = min(o, 1) (Pool) ; DMA out.
                # pair s of the group lands on psum rows s (top) and 32+s
                # (bottom), so ship the two row blocks separately.
                o = opool.tile([H, BW], BF16, tag="o", name=f"o_{G}_{c0}")
                nc.scalar.activation(
                    o[:, c0 : c0 + cw], pt[:], AF.Relu,
                    scale=1.0 / 6.0, bias=kcol,
                )
                oo = opool.tile([H, BW], BF16, tag="oo", name=f"oo_{G}_{c0}")
                nc.gpsimd.tensor_scalar_min(
                    oo[:, c0 : c0 + cw], o[:, c0 : c0 + cw], 1.0
                )
                nc.sync.dma_start(
                    out_d[2 * pb : 2 * pb + gsz, c0 : c0 + cw],
                    oo[0:gsz, c0 : c0 + cw],
                )
                nc.sync.dma_start(
                    out_d[2 * pb + gsz : 2 * pb + 2 * gsz, c0 : c0 + cw],
                    oo[32 : 32 + gsz, c0 : c0 + cw],
                )

            pending_fin = []
            pbase = 0

            for G, GSZ in enumerate(GROUP_SIZES):
                pc0 = ps.tile([H, CW0], F32, tag="pc0", name=f"pc0_{G}")
                pc1 = ps.tile([H, CW1], F32, tag="pcx", name=f"pc1_{G}")
                started0 = False
                started1 = [False]

                # proportionally interleaved scheme schedule (Bresenham)
                if GROUP_QUOTAS is not None:
                    NZg, NYg, NWg, NWPg = GROUP_QUOTAS[G]
                else:
                    NZg, NYg, NWg, NWPg = NZ, NY, NW, NWP
                quota = {"Z": NZg * GSZ / 32.0, "Y2": NYg * GSZ / 32.0,
                         "W2": NWg * GSZ / 32.0}
                counts = {k: 0 for k in quota}
                sched = []
                for s in range(GSZ):
                    k = max(quota, key=lambda k: quota[k] * (s + 1) / GSZ - counts[k])
                    sched.append(k)
                    counts[k] += 1
                NWG = counts["W2"]
                NWPG = min(NWPg, NWG)
                w2_idx = 0
                # last slot must not be W2 (deferred W2 work flushes before it
                # so the group's stop-flag matmul stays last)
                if sched[GSZ - 1] == "W2":
                    for si in range(GSZ - 2, -1, -1):
                        if sched[si] != "W2":
                            sched[GSZ - 1], sched[si] = sched[si], sched[GSZ - 1]
                            break
                deferred = []

                for s in range(GSZ):
                    p = pbase + s
                    scheme = sched[s]
                    # q = min(Ei * E2j, 1) = e^{min(u,0)}   (always DVE, 4x)
                    q = fq.tile([D, BW], BF16, tag="q", name=f"q_{G}_{s}")
                    nc.vector.tensor_scalar(
                        q[:], E2w[:, p : p + BW], EIS[:, p : p + 1],
                        1.0, OP.mult, OP.min,
                    )
                    # rs = relu(u)/al : DVE for Z slots, ACT otherwise
                    rs = fq.tile([D, BW], BF16, tag="rs", name=f"rs_{G}_{s}")
                    if scheme == "Z":
                        nc.vector.tensor_scalar(
                            rs[:], g2w[:, p : p + BW], gbs_s[:, p : p + 1],
                            0.0, OP.add, OP.max,
                        )
                    else:
                        nc.scalar.activation(
                            rs[:], g2w[:, p : p + BW], AF.Relu,
                            bias=gbs_s[:, p : p + 1],
                        )
                    def emit_mms(slot, movers, last):
                        nonlocal started0
                        pat1 = p1[:, H * slot : H * (slot + 1)]
                        for pt, c0, cw in ((pc0, 0, CW0), (pc1, CW0, CW1)):
                            for mi, mv in enumerate(movers):
                                nc.tensor.matmul(
                                    pt[:], pat1, mv[:, c0 : c0 + cw],
                                    start=(not started0 and c0 == 0 and mi == 0)
                                    or (not started1[0] and c0 == CW0 and mi == 0),
                                    stop=last and mi == len(movers) - 1,
                                    skip_group_check=True,
                                )
                                if c0 == CW0:
                                    started1[0] = True
                        started0 = True

                    if s == GSZ - 1:
                        # flush deferred W2 work so this slot's mms close psum
                        for emit in deferred:
                            emit()
                        deferred = []

                    if scheme == "W2":
                        f = fq.tile([D, BW], BF16, tag="f", name=f"f_{G}_{s}")
                        on_pool = NWPG > 0 and (
                            (w2_idx + 1) * NWPG // NWG > w2_idx * NWPG // NWG
                        )
                        w2_idx += 1

                        def emit_w2(f=f, q=q, rs=rs, slot=s, on_pool=on_pool):
                            if on_pool:
                                nc.gpsimd.tensor_tensor(f[:], q[:], rs[:], OP.add)
                            else:
                                nc.vector.tensor_tensor(f[:], q[:], rs[:], OP.add)
                            emit_mms(slot, (f,), False)

                        deferred.append(emit_w2)
                    else:
                        emit_mms(s, (q, rs), s == GSZ - 1)
                        # drain one deferred W2 behind this slot's ready work
                        if len(deferred) > 1:
                            deferred.pop(0)()
                    # emit previous group's finalize mid-stream so it doesn't
                    # head-of-line-block this group's producer ops
                    if s == 6 and pending_fin:
                        for fin in pending_fin:
                            finalize(*fin)
                        pending_fin = []
                pending_fin.append((G, pbase, GSZ, pc0, 0, CW0))
                pending_fin.append((G, pbase, GSZ, pc1, CW0, CW1))
                pbase += GSZ
            for fin in pending_fin:
                finalize(*fin)

    nc.compile()
    return nc


def _host_inputs(x, W_enc, b_enc, W1a, b1a, W1b, b1b, W2a, b2a, W2b, b2b):
    w = W2b[0].astype(np.float64)
    SW = float(w.sum())
    KB = (-LAM * AL * SW + float(b2b[0])) / 6.0 + 0.5

    import ml_dtypes

    lam_al_w = (LAM * AL * w).astype(np.float32)
    p1 = np.zeros((D, 32 * H), np.float32)
    for s in range(32):
        p1[0:H, H * s + s] = lam_al_w
        p1[H:D, H * s + 32 + s] = lam_al_w

    W_combo = (W1a.astype(np.float64) @ W_enc.astype(np.float64)).astype(
        np.float32
    )  # [H, RAW]
    b_combo = (W1a.astype(np.float64) @ b_enc.astype(np.float64)
               + b1a.astype(np.float64)).astype(np.float32)
    cpack = np.zeros((D, 5), np.float32)
    cpack[:, 1] = b1b
    cpack[:, 2] = np.concatenate([b2a, b2a])
    cpack[0:H, 3] = b_combo
    cpack[0:H, 4] = KB
    w2aT = np.ascontiguousarray(W2a.T)
    wcmbT = np.concatenate(
        [W_combo.T[k * D : (k + 1) * D] for k in range(4)], axis=1
    )  # [128, 4*64]
    wab = np.concatenate([wcmbT, w2aT, w2aT], axis=1)
    common = {
        "cpack": cpack,
        "wab": wab.astype(ml_dtypes.bfloat16),
        "w1bT": np.ascontiguousarray(W1b.T).astype(ml_dtypes.bfloat16),
        "p1": p1.astype(ml_dtypes.bfloat16),
    }
    in_maps = []
    for c in range(NCORES):
        m = dict(common)
        xr = np.roll(x, -ROWS * c, axis=0)[:XCOLS]
        xrT = np.ascontiguousarray(xr.T)  # [512, 1056]
        m["xT"] = np.concatenate(
            [xrT[k * D : (k + 1) * D] for k in range(4)], axis=1
        ).astype(ml_dtypes.bfloat16)
        in_maps.append(m)
    return in_maps


def _assemble(results):
    N = N_NODES
    O = np.zeros((N, N), np.float32)
    dd = np.arange(769)
    for c in range(NCORES):
        T = np.asarray(results[c]["out"][:, :769], np.float32)
        # rows grouped as (group, half, s) per GROUP_SIZES
        i_loc = np.empty(ROWS, np.int64)
        rbase = 0
        pbase = 0
        for gsz in GROUP_SIZES:
            sidx = np.arange(gsz)
            i_loc[rbase : rbase + gsz] = pbase + sidx
            i_loc[rbase + gsz : rbase + 2 * gsz] = pbase + sidx + 96
            rbase += 2 * gsz
            pbase += gsz
        gi = (ROWS * c + i_loc) % N
        cols = (gi[:, None] + dd[None, :]) % N
        O[gi[:, None], cols] = T
        O[cols, gi[:, None]] = T
    return O


def kernel(x, W_enc, b_enc, W1a, b1a, W1b, b1b, W2a, b2a, W2b, b2b):
    from concourse.bass_utils import run_bass_kernel_spmd

    global _compiled
    if _compiled is None:
        _compiled = _build_program()
    in_maps = _host_inputs(
        np.asarray(x, np.float32),
        np.asarray(W_enc, np.float32), np.asarray(b_enc, np.float32),
        np.asarray(W1a, np.float32), np.asarray(b1a, np.float32),
        np.asarray(W1b, np.float32), np.asarray(b1b, np.float32),
        np.asarray(W2a, np.float32), np.asarray(b2a, np.float32),
        np.asarray(W2b, np.float32), np.asarray(b2b, np.float32),
    )
    res = run_bass_kernel_spmd(_compiled, in_maps, list(range(NCORES)))
    return _assemble(res.results)


# revision 8
# speedup vs baseline: 1.0195x; 1.0195x over previous
"""Trainium2 Bass kernel v2 for nn_LCAMatrixModel (pairwise selu-MLP scoring).

o[i,j] = hardsigmoid( sum_h W2b[h]*selu(g[i,h]+g[j,h]+b2a[h]) + b2b ), symmetric.

Circulant band decomposition: each core owns 192 consecutive global rows
(core c: rows [192c, 192c+192)) of the band t[i, d] = o[i, (i+d) % N],
d in [0, 769) - exactly half the matrix. Inputs are host-rolled by 192c so
all cores run the same program on local rows [0, 192). Rows are processed
as 96 pairs (i, i+96) stacked on 128 partitions (2 x 64 h).

Per-pair math over the band window (u = g_i + g_j + b2a):
  sum_h w*selu(u) = lam*al*sum_h w*(rs + q) - lam*al*Sw,  with
  rs = relu(u)/al   and   q = e^{min(u,0)} = min(e^{g_i} * e^{g_j+b2a}, 1)
(the exp is SEPARABLE: E=e^g per node is precomputed, so the pairwise q is
just a multiply+min). o = clip(V/6 + 0.5, 0, 1). Every matmul uses the one
stationary pattern P1 = lam*al*w (g is stored pre-divided by al).

Engine balance via per-pair schemes (slot counts tunable):
  Z  : rs = DVE ts(add,max0);  q = DVE ts(mult,min1);  PE: P1@q + P1@rs
  Y2 : rs = ACT Relu(g2w+bias); q = DVE;               PE: P1@q + P1@rs
  W2 : rs = ACT; q = DVE; f = q+rs (DVE or Pool tt);   PE: P1@f  (half PE)
"""
import sys

sys.path.insert(0, "/opt/trn_rl_repo")

import numpy as np

N_NODES = 1536
RAW = 512
D = 128
H = 64
NCORES = 8
ROWS = N_NODES // NCORES          # 192 rows per core
PAIRS = ROWS // 2                 # 96 pairs (i, i+96)
GROUPS = 3                        # 3 groups of 32 pairs
BW = 772                          # padded band width (769 used)
GSPAN = 960                       # g2w width: max i (95) + BW + pad
XCOLS = 1056                      # g cols needed: GSPAN + 96 (bottom shift)
CW0, CW1 = 356, BW - 356          # psum chunk widths (356+416)
ECW = 352                         # encoder/prologue chunk width (3 x 352 = 1056)

LAM = 1.0507009873554805
AL = 1.6732632423543772

# per-group scheme quotas (fractions of 32): Z=DVE rs, Y2=ACT rs + 2mm,
# W2=ACT rs + f-combine + 1mm. Of W2, NWP_FR on Pool, rest DVE.
NZ_DEF = 15
NY_DEF = 3
NWP_DEF = 7
GROUP_SIZES = (32, 32, 32)
# per-group (NZ, NY, NW, NWP): front-load PE work, drain PE-light
GROUP_QUOTAS = None  # uniform per-group quotas

_compiled = None


def _build_program(NZ=NZ_DEF, NY=NY_DEF, NWP=NWP_DEF):
    import concourse.bacc as bacc
    import concourse.mybir as mybir
    import concourse.tile as tile

    F32 = mybir.dt.float32
    BF16 = mybir.dt.bfloat16
    AF = mybir.ActivationFunctionType
    OP = mybir.AluOpType

    NW = 32 - NZ - NY

    nc = bacc.Bacc("TRN2", target_bir_lowering=False, debug=False)

    # ---- DRAM I/O ----
    xT_d = nc.dram_tensor("xT", [D, 4 * XCOLS], BF16, kind="ExternalInput")
    cpack_d = nc.dram_tensor("cpack", [D, 5], F32, kind="ExternalInput")
    wab_d = nc.dram_tensor("wab", [D, 6 * H], BF16, kind="ExternalInput")
    w1bT_d = nc.dram_tensor("w1bT", [H, D], BF16, kind="ExternalInput")
    p1_d = nc.dram_tensor("p1", [D, 32 * H], BF16, kind="ExternalInput")
    out_d = nc.dram_tensor("out", [ROWS, BW], BF16, kind="ExternalOutput")

    with tile.TileContext(nc) as tc:
        with (
            tc.tile_pool(name="cst", bufs=1) as cst,
            tc.tile_pool(name="enc", bufs=1) as enc,
            tc.tile_pool(name="pre", bufs=3) as pre,
            tc.tile_pool(name="fq", bufs=28) as fq,
            tc.tile_pool(name="op", bufs=6) as opool,
            tc.tile_pool(name="ps", bufs=2, space="PSUM") as ps,
            tc.tile_pool(name="psp", bufs=4, space="PSUM") as psp,
        ):
            # ---- input DMAs (x first - encoder needs it asap) ----
            xt = enc.tile([D, 4 * XCOLS], BF16)
            nc.scalar.dma_start(xt[:, :], xT_d[:, :])
            cpack = cst.tile([D, 5], F32)
            nc.sync.dma_start(cpack[:], cpack_d[:])
            wab = cst.tile([D, 6 * H], BF16)
            nc.sync.dma_start(wab[:], wab_d[:])
            w1bT = cst.tile([H, D], BF16)
            nc.sync.dma_start(w1bT[:, :], w1bT_d[:])
            p1 = cst.tile([D, 32 * H], BF16)
            nc.sync.dma_start(p1[:], p1_d[:])
            b1b = cpack[:, 1:2]
            b2a2 = cpack[:, 2:3]
            bcmb = cpack[0:H, 3:4]
            kcol = cpack[0:H, 4:5]
            wcmb = wab[:, 0 : 4 * H]       # W1a@W_enc, 4 k-tiles of [128,64]
            w2aT = wab[:, 4 * H : 6 * H]
            NCH = XCOLS // ECW  # 3 chunks of 352

            # ---- selu helper: v = psum+b (bf16), then 4x-mode DVE ops ----
            def selu_from_psum(out_ap, pa, b_raw, p, nm):
                v = pre.tile([p, ECW], BF16, tag="selv", name=f"v_{nm}")
                nc.vector.tensor_scalar(v[:], pa, b_raw, 0.0, OP.add, OP.add)
                r = pre.tile([p, ECW], BF16, tag="selr", name=f"r_{nm}")
                nc.vector.tensor_scalar(r[:], v[:], 0.0, LAM, OP.max, OP.mult)
                m = pre.tile([p, ECW], BF16, tag="selm", name=f"m_{nm}")
                nc.vector.tensor_scalar_min(m[:], v[:], 0.0)
                e = pre.tile([p, ECW], BF16, tag="sele", name=f"e_{nm}")
                nc.scalar.activation(e[:], m[:], AF.Exp)
                t = pre.tile([p, ECW], BF16, tag="selt", name=f"t_{nm}")
                nc.vector.tensor_scalar(t[:], e[:], LAM * AL, -LAM * AL, OP.mult, OP.add)
                nc.vector.tensor_tensor(out_ap, r[:], t[:], OP.add)

            # ---- wavefront emission over (layer, chunk): keeps every
            # engine queue supplied while chains pipeline ----
            a1T = enc.tile([H, XCOLS], BF16)
            hT = enc.tile([D, XCOLS], BF16)
            g2dup = cst.tile([D, XCOLS], F32)
            g2w = cst.tile([D, GSPAN], BF16)
            E2w = cst.tile([D, GSPAN], BF16)
            gbs_raw = cst.tile([D, PAIRS], F32)
            gbs_s = cst.tile([D, PAIRS], F32)
            EIS = cst.tile([D, PAIRS], F32)

            def do_a1(c):
                # fused encoder+first layer: a1pre = x @ (W1a@W_enc).T
                sl = slice(c * ECW, (c + 1) * ECW)
                pa = psp.tile([H, ECW], F32, tag="pp", name=f"pa_{c}")
                for k in range(4):
                    nc.tensor.matmul(
                        pa[:],
                        wcmb[:, k * H : (k + 1) * H],
                        xt[:, k * XCOLS + c * ECW : k * XCOLS + (c + 1) * ECW],
                        start=(k == 0),
                        stop=(k == 3),
                    )
                selu_from_psum(a1T[:, sl], pa[:], bcmb, H, f"a{c}")

            def do_h(c):
                sl = slice(c * ECW, (c + 1) * ECW)
                ph = psp.tile([D, ECW], F32, tag="pp", name=f"ph_{c}")
                nc.tensor.matmul(ph[:], w1bT[:, :], a1T[:, sl], start=True, stop=True)
                selu_from_psum(hT[:, sl], ph[:], b1b, D, f"h{c}")

            def do_g(c):
                sl = slice(c * ECW, (c + 1) * ECW)
                pg = psp.tile([D, ECW], F32, tag="pp", name=f"pg_{c}")
                nc.tensor.matmul(pg[:], w2aT, hT[:, sl], start=True, stop=True)
                nc.scalar.activation(g2dup[:, sl], pg[:], AF.Copy)
                # build the g2w/E2w pieces this chunk enables (top: same cols,
                # bottom: cols shifted by -96), so the main loop can start
                # right after the last chunk
                t0, t1 = c * ECW, min((c + 1) * ECW, GSPAN)
                if t1 > t0:
                    nc.gpsimd.tensor_scalar_mul(
                        g2w[0:H, t0:t1], g2dup[0:H, t0:t1], 1.0 / AL
                    )
                    nc.scalar.activation(
                        E2w[0:H, t0:t1], g2dup[0:H, t0:t1], AF.Exp,
                        bias=b2a2[0:H, :],
                    )
                b0 = max(c * ECW - 96, 0)
                b1 = min((c + 1) * ECW - 96, GSPAN)
                if b1 > b0:
                    nc.gpsimd.tensor_scalar_mul(
                        g2w[H:D, b0:b1], g2dup[H:D, 96 + b0 : 96 + b1], 1.0 / AL
                    )
                    nc.scalar.activation(
                        E2w[H:D, b0:b1], g2dup[H:D, 96 + b0 : 96 + b1], AF.Exp,
                        bias=b2a2[H:D, :],
                    )
                if c == 0:
                    nc.gpsimd.tensor_copy(gbs_raw[0:H, :], g2dup[0:H, 0:PAIRS])
                    nc.gpsimd.tensor_copy(
                        gbs_raw[H:D, :], g2dup[H:D, 96 : 96 + PAIRS]
                    )
                    nc.vector.tensor_scalar(
                        gbs_s[:], gbs_raw[:], b2a2, 1.0 / AL, OP.add, OP.mult
                    )
                    nc.scalar.activation(EIS[:], gbs_raw[:], AF.Exp)

            layers = (do_a1, do_h, do_g)
            for step in range(len(layers) + NCH - 1):
                for c in range(NCH):
                    li = step - c
                    if 0 <= li < len(layers):
                        layers[li](c)

            # ================= main pairwise loop =================
            def finalize(G, pb, gsz, pt, c0, cw):
                # o = Relu(psum/6 + kb) ; out = min(o, 1) ; DMA out.
                # pair s of the group lands on psum rows s (top) and 32+s
                # (bottom). The min runs on Pool mid-stream but on DVE for
                # the last group (DVE is idle by then; Pool's slower op sits
                # on the kernel's critical tail).
                o = opool.tile([H, BW], BF16, tag="o", name=f"o_{G}_{c0}")
                nc.scalar.activation(
                    o[:, c0 : c0 + cw], pt[:], AF.Relu,
                    scale=1.0 / 6.0, bias=kcol,
                )
                oo = opool.tile([H, BW], BF16, tag="oo", name=f"oo_{G}_{c0}")
                if G == len(GROUP_SIZES) - 1:
                    nc.vector.tensor_scalar_min(
                        oo[:, c0 : c0 + cw], o[:, c0 : c0 + cw], 1.0
                    )
                else:
                    nc.gpsimd.tensor_scalar_min(
                        oo[:, c0 : c0 + cw], o[:, c0 : c0 + cw], 1.0
                    )
                if gsz == 32:
                    # both row halves are contiguous: one descriptor
                    nc.sync.dma_start(
                        out_d[2 * pb : 2 * pb + 64, c0 : c0 + cw],
                        oo[:, c0 : c0 + cw],
                    )
                else:
                    nc.sync.dma_start(
                        out_d[2 * pb : 2 * pb + gsz, c0 : c0 + cw],
                        oo[0:gsz, c0 : c0 + cw],
                    )
                    nc.sync.dma_start(
                        out_d[2 * pb + gsz : 2 * pb + 2 * gsz, c0 : c0 + cw],
                        oo[32 : 32 + gsz, c0 : c0 + cw],
                    )

            pending_fin = []
            pbase = 0

            for G, GSZ in enumerate(GROUP_SIZES):
                pc0 = ps.tile([H, CW0], F32, tag="pc0", name=f"pc0_{G}")
                pc1 = ps.tile([H, CW1], F32, tag="pcx", name=f"pc1_{G}")
                started0 = False
                started1 = [False]

                # proportionally interleaved scheme schedule (Bresenham)
                if GROUP_QUOTAS is not None:
                    NZg, NYg, NWg, NWPg = GROUP_QUOTAS[G]
                else:
                    NZg, NYg, NWg, NWPg = NZ, NY, NW, NWP
                quota = {"Z": NZg * GSZ / 32.0, "Y2": NYg * GSZ / 32.0,
                         "W2": NWg * GSZ / 32.0}
                counts = {k: 0 for k in quota}
                sched = []
                for s in range(GSZ):
                    k = max(quota, key=lambda k: quota[k] * (s + 1) / GSZ - counts[k])
                    sched.append(k)
                    counts[k] += 1
                NWG = counts["W2"]
                NWPG = min(NWPg, NWG)
                w2_idx = 0
                # last slot must not be W2 (deferred W2 work flushes before it
                # so the group's stop-flag matmul stays last)
                if sched[GSZ - 1] == "W2":
                    for si in range(GSZ - 2, -1, -1):
                        if sched[si] != "W2":
                            sched[GSZ - 1], sched[si] = sched[si], sched[GSZ - 1]
                            break
                deferred = []

                for s in range(GSZ):
                    p = pbase + s
                    scheme = sched[s]
                    # q = min(Ei * E2j, 1) = e^{min(u,0)}   (always DVE, 4x)
                    q = fq.tile([D, BW], BF16, tag="q", name=f"q_{G}_{s}")
                    nc.vector.tensor_scalar(
                        q[:], E2w[:, p : p + BW], EIS[:, p : p + 1],
                        1.0, OP.mult, OP.min,
                    )
                    # rs = relu(u)/al : DVE for Z slots, ACT otherwise
                    rs = fq.tile([D, BW], BF16, tag="rs", name=f"rs_{G}_{s}")
                    if scheme == "Z":
                        nc.vector.tensor_scalar(
                            rs[:], g2w[:, p : p + BW], gbs_s[:, p : p + 1],
                            0.0, OP.add, OP.max,
                        )
                    else:
                        nc.scalar.activation(
                            rs[:], g2w[:, p : p + BW], AF.Relu,
                            bias=gbs_s[:, p : p + 1],
                        )
                    def emit_mms(slot, movers, last):
                        nonlocal started0
                        pat1 = p1[:, H * slot : H * (slot + 1)]
                        for pt, c0, cw in ((pc0, 0, CW0), (pc1, CW0, CW1)):
                            for mi, mv in enumerate(movers):
                                nc.tensor.matmul(
                                    pt[:], pat1, mv[:, c0 : c0 + cw],
                                    start=(not started0 and c0 == 0 and mi == 0)
                                    or (not started1[0] and c0 == CW0 and mi == 0),
                                    stop=last and mi == len(movers) - 1,
                                    skip_group_check=True,
                                )
                                if c0 == CW0:
                                    started1[0] = True
                        started0 = True

                    if s == GSZ - 1:
                        # flush deferred W2 work so this slot's mms close psum
                        for emit in deferred:
                            emit()
                        deferred = []

                    if scheme == "W2":
                        f = fq.tile([D, BW], BF16, tag="f", name=f"f_{G}_{s}")
                        on_pool = NWPG > 0 and (
                            (w2_idx + 1) * NWPG // NWG > w2_idx * NWPG // NWG
                        )
                        w2_idx += 1

                        def emit_w2(f=f, q=q, rs=rs, slot=s, on_pool=on_pool):
                            if on_pool:
                                nc.gpsimd.tensor_tensor(f[:], q[:], rs[:], OP.add)
                            else:
                                nc.vector.tensor_tensor(f[:], q[:], rs[:], OP.add)
                            emit_mms(slot, (f,), False)

                        deferred.append(emit_w2)
                    else:
                        emit_mms(s, (q, rs), s == GSZ - 1)
                        # drain one deferred W2 behind this slot's ready work
                        if len(deferred) > 1:
                            deferred.pop(0)()
                    # emit previous group's finalize mid-stream so it doesn't
                    # head-of-line-block this group's producer ops
                    if s == 6 and pending_fin:
                        for fin in pending_fin:
                            finalize(*fin)
                        pending_fin = []
                pending_fin.append((G, pbase, GSZ, pc0, 0, CW0))
                pending_fin.append((G, pbase, GSZ, pc1, CW0, CW1))
                pbase += GSZ
            for fin in pending_fin:
                finalize(*fin)

    nc.compile()
    return nc


def _host_inputs(x, W_enc, b_enc, W1a, b1a, W1b, b1b, W2a, b2a, W2b, b2b):
    w = W2b[0].astype(np.float64)
    SW = float(w.sum())
    KB = (-LAM * AL * SW + float(b2b[0])) / 6.0 + 0.5

    import ml_dtypes

    lam_al_w = (LAM * AL * w).astype(np.float32)
    p1 = np.zeros((D, 32 * H), np.float32)
    for s in range(32):
        p1[0:H, H * s + s] = lam_al_w
        p1[H:D, H * s + 32 + s] = lam_al_w

    W_combo = (W1a.astype(np.float64) @ W_enc.astype(np.float64)).astype(
        np.float32
    )  # [H, RAW]
    b_combo = (W1a.astype(np.float64) @ b_enc.astype(np.float64)
               + b1a.astype(np.float64)).astype(np.float32)
    cpack = np.zeros((D, 5), np.float32)
    cpack[:, 1] = b1b
    cpack[:, 2] = np.concatenate([b2a, b2a])
    cpack[0:H, 3] = b_combo
    cpack[0:H, 4] = KB
    w2aT = np.ascontiguousarray(W2a.T)
    wcmbT = np.concatenate(
        [W_combo.T[k * D : (k + 1) * D] for k in range(4)], axis=1
    )  # [128, 4*64]
    wab = np.concatenate([wcmbT, w2aT, w2aT], axis=1)
    common = {
        "cpack": cpack,
        "wab": wab.astype(ml_dtypes.bfloat16),
        "w1bT": np.ascontiguousarray(W1b.T).astype(ml_dtypes.bfloat16),
        "p1": p1.astype(ml_dtypes.bfloat16),
    }
    in_maps = []
    for c in range(NCORES):
        m = dict(common)
        xr = np.roll(x, -ROWS * c, axis=0)[:XCOLS]
        xrT = np.ascontiguousarray(xr.T)  # [512, 1056]
        m["xT"] = np.concatenate(
            [xrT[k * D : (k + 1) * D] for k in range(4)], axis=1
        ).astype(ml_dtypes.bfloat16)
        in_maps.append(m)
    return in_maps


def _assemble(results):
    N = N_NODES
    O = np.zeros((N, N), np.float32)
    dd = np.arange(769)
    for c in range(NCORES):
        T = np.asarray(results[c]["out"][:, :769], np.float32)
        # rows grouped as (group, half, s) per GROUP_SIZES
        i_loc = np.empty(ROWS, np.int64)
        rbase = 0
        pbase = 0
        for gsz in GROUP_SIZES:
            sidx = np.arange(gsz)
            i_loc[rbase : rbase + gsz] = pbase + sidx
            i_loc[rbase + gsz : rbase + 2 * gsz] = pbase + sidx + 96
            rbase += 2 * gsz
            pbase += gsz
        gi = (ROWS * c + i_loc) % N
        cols = (gi[:, None] + dd[None, :]) % N
        O[gi[:, None], cols] = T
        O[cols, gi[:, None]] = T
    return O


def kernel(x, W_enc, b_enc, W1a, b1a, W1b, b1b, W2a, b2a, W2b, b2b):
    from concourse.bass_utils import run_bass_kernel_spmd

    global _compiled
    if _compiled is None:
        _compiled = _build_program()
    in_maps = _host_inputs(
        np.asarray(x, np.float32),
        np.asarray(W_enc, np.float32), np.asarray(b_enc, np.float32),
        np.asarray(W1a, np.float32), np.asarray(b1a, np.float32),
        np.asarray(W1b, np.float32), np.asarray(b1b, np.float32),
        np.asarray(W2a, np.float32), np.asarray(b2a, np.float32),
        np.asarray(W2b, np.float32), np.asarray(b2b, np.float32),
    )
    res = run_bass_kernel_spmd(_compiled, in_maps, list(range(NCORES)))
    return _assemble(res.results)


# revision 9
# speedup vs baseline: 1.0283x; 1.0086x over previous
"""Trainium2 Bass kernel v2 for nn_LCAMatrixModel (pairwise selu-MLP scoring).

o[i,j] = hardsigmoid( sum_h W2b[h]*selu(g[i,h]+g[j,h]+b2a[h]) + b2b ), symmetric.

Circulant band decomposition: each core owns 192 consecutive global rows
(core c: rows [192c, 192c+192)) of the band t[i, d] = o[i, (i+d) % N],
d in [0, 769) - exactly half the matrix. Inputs are host-rolled by 192c so
all cores run the same program on local rows [0, 192). Rows are processed
as 96 pairs (i, i+96) stacked on 128 partitions (2 x 64 h).

Per-pair math over the band window (u = g_i + g_j + b2a):
  sum_h w*selu(u) = lam*al*sum_h w*(rs + q) - lam*al*Sw,  with
  rs = relu(u)/al   and   q = e^{min(u,0)} = min(e^{g_i} * e^{g_j+b2a}, 1)
(the exp is SEPARABLE: E=e^g per node is precomputed, so the pairwise q is
just a multiply+min). o = clip(V/6 + 0.5, 0, 1). Every matmul uses the one
stationary pattern P1 = lam*al*w (g is stored pre-divided by al).

Engine balance via per-pair schemes (slot counts tunable):
  Z  : rs = DVE ts(add,max0);  q = DVE ts(mult,min1);  PE: P1@q + P1@rs
  Y2 : rs = ACT Relu(g2w+bias); q = DVE;               PE: P1@q + P1@rs
  W2 : rs = ACT; q = DVE; f = q+rs (DVE or Pool tt);   PE: P1@f  (half PE)
"""
import sys

sys.path.insert(0, "/opt/trn_rl_repo")

import numpy as np

N_NODES = 1536
RAW = 512
D = 128
H = 64
NCORES = 8
ROWS = N_NODES // NCORES          # 192 rows per core
PAIRS = ROWS // 2                 # 96 pairs (i, i+96)
GROUPS = 3                        # 3 groups of 32 pairs
BW = 772                          # padded band width (769 used)
GSPAN = 960                       # g2w width: max i (95) + BW + pad
XCOLS = 1056                      # g cols needed: GSPAN + 96 (bottom shift)
CW0, CW1 = 356, BW - 356          # psum chunk widths (356+416)
ECW = 352                         # encoder/prologue chunk width (3 x 352 = 1056)

LAM = 1.0507009873554805
AL = 1.6732632423543772

# per-group scheme quotas (fractions of 32): Z=DVE rs, Y2=ACT rs + 2mm,
# W2=ACT rs + f-combine + 1mm. Of W2, NWP_FR on Pool, rest DVE.
NZ_DEF = 15
NY_DEF = 3
NWP_DEF = 7
GROUP_SIZES = (32, 32, 32)
# per-group (NZ, NY, NW, NWP): front-load PE work, drain PE-light
GROUP_QUOTAS = None  # uniform per-group quotas

_compiled = None


def _build_program(NZ=NZ_DEF, NY=NY_DEF, NWP=NWP_DEF):
    import concourse.bacc as bacc
    import concourse.mybir as mybir
    import concourse.tile as tile

    F32 = mybir.dt.float32
    BF16 = mybir.dt.bfloat16
    AF = mybir.ActivationFunctionType
    OP = mybir.AluOpType

    NW = 32 - NZ - NY

    nc = bacc.Bacc("TRN2", target_bir_lowering=False, debug=False)

    # ---- DRAM I/O ----
    xT_d = nc.dram_tensor("xT", [D, 4 * XCOLS], BF16, kind="ExternalInput")
    cpack_d = nc.dram_tensor("cpack", [D, 5], F32, kind="ExternalInput")
    wab_d = nc.dram_tensor("wab", [D, 6 * H], BF16, kind="ExternalInput")
    w1bT_d = nc.dram_tensor("w1bT", [H, D], BF16, kind="ExternalInput")
    p1_d = nc.dram_tensor("p1", [D, 32 * H], BF16, kind="ExternalInput")
    out_d = nc.dram_tensor("out", [ROWS, BW], BF16, kind="ExternalOutput")

    with tile.TileContext(nc) as tc:
        with (
            tc.tile_pool(name="cst", bufs=1) as cst,
            tc.tile_pool(name="enc", bufs=1) as enc,
            tc.tile_pool(name="pre", bufs=3) as pre,
            tc.tile_pool(name="fq", bufs=28) as fq,
            tc.tile_pool(name="op", bufs=6) as opool,
            tc.tile_pool(name="ps", bufs=2, space="PSUM") as ps,
            tc.tile_pool(name="psp", bufs=4, space="PSUM") as psp,
        ):
            # ---- input DMAs (x first - encoder needs it asap) ----
            xt = enc.tile([D, 4 * XCOLS], BF16)
            nc.scalar.dma_start(xt[:, :], xT_d[:, :])
            cpack = cst.tile([D, 5], F32)
            nc.sync.dma_start(cpack[:], cpack_d[:])
            wab = cst.tile([D, 6 * H], BF16)
            nc.sync.dma_start(wab[:], wab_d[:])
            w1bT = cst.tile([H, D], BF16)
            nc.sync.dma_start(w1bT[:, :], w1bT_d[:])
            p1 = cst.tile([D, 32 * H], BF16)
            nc.sync.dma_start(p1[:], p1_d[:])
            b1b = cpack[:, 1:2]
            b2a2 = cpack[:, 2:3]
            bcmb = cpack[0:H, 3:4]
            kcol = cpack[0:H, 4:5]
            wcmb = wab[:, 0 : 4 * H]       # W1a@W_enc, 4 k-tiles of [128,64]
            w2aT = wab[:, 4 * H : 6 * H]
            NCH = XCOLS // ECW  # 3 chunks of 352

            # ---- selu helper: v = psum+b (bf16); negative branch via
            # min(lam*al*(e^v - 1), 0) so exp needs no pre-clamp; final
            # clamp+add fused into one scalar_tensor_tensor ----
            def selu_from_psum(out_ap, pa, b_raw, p, nm):
                v = pre.tile([p, ECW], BF16, tag="selv", name=f"v_{nm}")
                nc.vector.tensor_scalar(v[:], pa, b_raw, 0.0, OP.add, OP.add)
                r = pre.tile([p, ECW], BF16, tag="selr", name=f"r_{nm}")
                nc.vector.tensor_scalar(r[:], v[:], 0.0, LAM, OP.max, OP.mult)
                e = pre.tile([p, ECW], BF16, tag="sele", name=f"e_{nm}")
                nc.scalar.activation(e[:], v[:], AF.Exp)
                t = pre.tile([p, ECW], BF16, tag="selt", name=f"t_{nm}")
                nc.vector.tensor_scalar(t[:], e[:], LAM * AL, -LAM * AL, OP.mult, OP.add)
                nc.vector.scalar_tensor_tensor(
                    out_ap, t[:], 0.0, r[:], OP.min, OP.add
                )

            # ---- wavefront emission over (layer, chunk): keeps every
            # engine queue supplied while chains pipeline ----
            a1T = enc.tile([H, XCOLS], BF16)
            hT = enc.tile([D, XCOLS], BF16)
            g2dup = cst.tile([D, XCOLS], F32)
            g2w = cst.tile([D, GSPAN], BF16)
            E2w = cst.tile([D, GSPAN], BF16)
            gbs_raw = cst.tile([D, PAIRS], F32)
            gbs_s = cst.tile([D, PAIRS], F32)
            EIS = cst.tile([D, PAIRS], F32)

            def do_a1(c):
                # fused encoder+first layer: a1pre = x @ (W1a@W_enc).T
                sl = slice(c * ECW, (c + 1) * ECW)
                pa = psp.tile([H, ECW], F32, tag="pp", name=f"pa_{c}")
                for k in range(4):
                    nc.tensor.matmul(
                        pa[:],
                        wcmb[:, k * H : (k + 1) * H],
                        xt[:, k * XCOLS + c * ECW : k * XCOLS + (c + 1) * ECW],
                        start=(k == 0),
                        stop=(k == 3),
                    )
                selu_from_psum(a1T[:, sl], pa[:], bcmb, H, f"a{c}")

            def do_h(c):
                sl = slice(c * ECW, (c + 1) * ECW)
                ph = psp.tile([D, ECW], F32, tag="pp", name=f"ph_{c}")
                nc.tensor.matmul(ph[:], w1bT[:, :], a1T[:, sl], start=True, stop=True)
                selu_from_psum(hT[:, sl], ph[:], b1b, D, f"h{c}")

            def do_g(c):
                sl = slice(c * ECW, (c + 1) * ECW)
                pg = psp.tile([D, ECW], F32, tag="pp", name=f"pg_{c}")
                nc.tensor.matmul(pg[:], w2aT, hT[:, sl], start=True, stop=True)
                nc.scalar.activation(g2dup[:, sl], pg[:], AF.Copy)
                # build the g2w/E2w pieces this chunk enables (top: same cols,
                # bottom: cols shifted by -96), so the main loop can start
                # right after the last chunk
                t0, t1 = c * ECW, min((c + 1) * ECW, GSPAN)
                if t1 > t0:
                    nc.gpsimd.tensor_scalar_mul(
                        g2w[0:H, t0:t1], g2dup[0:H, t0:t1], 1.0 / AL
                    )
                    nc.scalar.activation(
                        E2w[0:H, t0:t1], g2dup[0:H, t0:t1], AF.Exp,
                        bias=b2a2[0:H, :],
                    )
                b0 = max(c * ECW - 96, 0)
                b1 = min((c + 1) * ECW - 96, GSPAN)
                if b1 > b0:
                    nc.gpsimd.tensor_scalar_mul(
                        g2w[H:D, b0:b1], g2dup[H:D, 96 + b0 : 96 + b1], 1.0 / AL
                    )
                    nc.scalar.activation(
                        E2w[H:D, b0:b1], g2dup[H:D, 96 + b0 : 96 + b1], AF.Exp,
                        bias=b2a2[H:D, :],
                    )
                if c == 0:
                    nc.gpsimd.tensor_copy(gbs_raw[0:H, :], g2dup[0:H, 0:PAIRS])
                    nc.gpsimd.tensor_copy(
                        gbs_raw[H:D, :], g2dup[H:D, 96 : 96 + PAIRS]
                    )
                    nc.vector.tensor_scalar(
                        gbs_s[:], gbs_raw[:], b2a2, 1.0 / AL, OP.add, OP.mult
                    )
                    nc.scalar.activation(EIS[:], gbs_raw[:], AF.Exp)

            layers = (do_a1, do_h, do_g)
            for step in range(len(layers) + NCH - 1):
                for c in range(NCH):
                    li = step - c
                    if 0 <= li < len(layers):
                        layers[li](c)

            # ================= main pairwise loop =================
            def finalize(G, pb, gsz, pt, c0, cw):
                # o = Relu(psum/6 + kb) ; out = min(o, 1) ; DMA out.
                # pair s of the group lands on psum rows s (top) and 32+s
                # (bottom). The min runs on Pool mid-stream but on DVE for
                # the last group (DVE is idle by then; Pool's slower op sits
                # on the kernel's critical tail).
                o = opool.tile([H, BW], BF16, tag="o", name=f"o_{G}_{c0}")
                nc.scalar.activation(
                    o[:, c0 : c0 + cw], pt[:], AF.Relu,
                    scale=1.0 / 6.0, bias=kcol,
                )
                oo = opool.tile([H, BW], BF16, tag="oo", name=f"oo_{G}_{c0}")
                if G == len(GROUP_SIZES) - 1:
                    nc.vector.tensor_scalar_min(
                        oo[:, c0 : c0 + cw], o[:, c0 : c0 + cw], 1.0
                    )
                else:
                    nc.gpsimd.tensor_scalar_min(
                        oo[:, c0 : c0 + cw], o[:, c0 : c0 + cw], 1.0
                    )
                if gsz == 32:
                    # both row halves are contiguous: one descriptor
                    nc.sync.dma_start(
                        out_d[2 * pb : 2 * pb + 64, c0 : c0 + cw],
                        oo[:, c0 : c0 + cw],
                    )
                else:
                    nc.sync.dma_start(
                        out_d[2 * pb : 2 * pb + gsz, c0 : c0 + cw],
                        oo[0:gsz, c0 : c0 + cw],
                    )
                    nc.sync.dma_start(
                        out_d[2 * pb + gsz : 2 * pb + 2 * gsz, c0 : c0 + cw],
                        oo[32 : 32 + gsz, c0 : c0 + cw],
                    )

            pending_fin = []
            pbase = 0

            for G, GSZ in enumerate(GROUP_SIZES):
                pc0 = ps.tile([H, CW0], F32, tag="pc0", name=f"pc0_{G}")
                pc1 = ps.tile([H, CW1], F32, tag="pcx", name=f"pc1_{G}")
                started0 = False
                started1 = [False]

                # proportionally interleaved scheme schedule (Bresenham)
                if GROUP_QUOTAS is not None:
                    NZg, NYg, NWg, NWPg = GROUP_QUOTAS[G]
                else:
                    NZg, NYg, NWg, NWPg = NZ, NY, NW, NWP
                quota = {"Z": NZg * GSZ / 32.0, "Y2": NYg * GSZ / 32.0,
                         "W2": NWg * GSZ / 32.0}
                counts = {k: 0 for k in quota}
                sched = []
                for s in range(GSZ):
                    k = max(quota, key=lambda k: quota[k] * (s + 1) / GSZ - counts[k])
                    sched.append(k)
                    counts[k] += 1
                NWG = counts["W2"]
                NWPG = min(NWPg, NWG)
                w2_idx = 0
                # last slot must not be W2 (deferred W2 work flushes before it
                # so the group's stop-flag matmul stays last)
                if sched[GSZ - 1] == "W2":
                    for si in range(GSZ - 2, -1, -1):
                        if sched[si] != "W2":
                            sched[GSZ - 1], sched[si] = sched[si], sched[GSZ - 1]
                            break
                deferred = []

                for s in range(GSZ):
                    p = pbase + s
                    scheme = sched[s]
                    # q = min(Ei * E2j, 1) = e^{min(u,0)}   (always DVE, 4x)
                    q = fq.tile([D, BW], BF16, tag="q", name=f"q_{G}_{s}")
                    nc.vector.tensor_scalar(
                        q[:], E2w[:, p : p + BW], EIS[:, p : p + 1],
                        1.0, OP.mult, OP.min,
                    )
                    # rs = relu(u)/al : DVE for Z slots, ACT otherwise
                    rs = fq.tile([D, BW], BF16, tag="rs", name=f"rs_{G}_{s}")
                    if scheme == "Z":
                        nc.vector.tensor_scalar(
                            rs[:], g2w[:, p : p + BW], gbs_s[:, p : p + 1],
                            0.0, OP.add, OP.max,
                        )
                    else:
                        nc.scalar.activation(
                            rs[:], g2w[:, p : p + BW], AF.Relu,
                            bias=gbs_s[:, p : p + 1],
                        )
                    def emit_mms(slot, movers, last):
                        nonlocal started0
                        pat1 = p1[:, H * slot : H * (slot + 1)]
                        for pt, c0, cw in ((pc0, 0, CW0), (pc1, CW0, CW1)):
                            for mi, mv in enumerate(movers):
                                nc.tensor.matmul(
                                    pt[:], pat1, mv[:, c0 : c0 + cw],
                                    start=(not started0 and c0 == 0 and mi == 0)
                                    or (not started1[0] and c0 == CW0 and mi == 0),
                                    stop=last and mi == len(movers) - 1,
                                    skip_group_check=True,
                                )
                                if c0 == CW0:
                                    started1[0] = True
                        started0 = True

                    if s == GSZ - 1:
                        # flush deferred W2 work so this slot's mms close psum
                        for emit in deferred:
                            emit()
                        deferred = []

                    if scheme == "W2":
                        f = fq.tile([D, BW], BF16, tag="f", name=f"f_{G}_{s}")
                        on_pool = NWPG > 0 and (
                            (w2_idx + 1) * NWPG // NWG > w2_idx * NWPG // NWG
                        )
                        w2_idx += 1

                        def emit_w2(f=f, q=q, rs=rs, slot=s, on_pool=on_pool):
                            if on_pool:
                                nc.gpsimd.tensor_tensor(f[:], q[:], rs[:], OP.add)
                            else:
                                nc.vector.tensor_tensor(f[:], q[:], rs[:], OP.add)
                            emit_mms(slot, (f,), False)

                        deferred.append(emit_w2)
                    else:
                        emit_mms(s, (q, rs), s == GSZ - 1)
                        # drain one deferred W2 behind this slot's ready work
                        if len(deferred) > 1:
                            deferred.pop(0)()
                    # emit previous group's finalize mid-stream so it doesn't
                    # head-of-line-block this group's producer ops
                    if s == 6 and pending_fin:
                        for fin in pending_fin:
                            finalize(*fin)
                        pending_fin = []
                pending_fin.append((G, pbase, GSZ, pc0, 0, CW0))
                pending_fin.append((G, pbase, GSZ, pc1, CW0, CW1))
                pbase += GSZ
            for fin in pending_fin:
                finalize(*fin)

    nc.compile()
    return nc


def _host_inputs(x, W_enc, b_enc, W1a, b1a, W1b, b1b, W2a, b2a, W2b, b2b):
    w = W2b[0].astype(np.float64)
    SW = float(w.sum())
    KB = (-LAM * AL * SW + float(b2b[0])) / 6.0 + 0.5

    import ml_dtypes

    lam_al_w = (LAM * AL * w).astype(np.float32)
    p1 = np.zeros((D, 32 * H), np.float32)
    for s in range(32):
        p1[0:H, H * s + s] = lam_al_w
        p1[H:D, H * s + 32 + s] = lam_al_w

    W_combo = (W1a.astype(np.float64) @ W_enc.astype(np.float64)).astype(
        np.float32
    )  # [H, RAW]
    b_combo = (W1a.astype(np.float64) @ b_enc.astype(np.float64)
               + b1a.astype(np.float64)).astype(np.float32)
    cpack = np.zeros((D, 5), np.float32)
    cpack[:, 1] = b1b
    cpack[:, 2] = np.concatenate([b2a, b2a])
    cpack[0:H, 3] = b_combo
    cpack[0:H, 4] = KB
    w2aT = np.ascontiguousarray(W2a.T)
    wcmbT = np.concatenate(
        [W_combo.T[k * D : (k + 1) * D] for k in range(4)], axis=1
    )  # [128, 4*64]
    wab = np.concatenate([wcmbT, w2aT, w2aT], axis=1)
    common = {
        "cpack": cpack,
        "wab": wab.astype(ml_dtypes.bfloat16),
        "w1bT": np.ascontiguousarray(W1b.T).astype(ml_dtypes.bfloat16),
        "p1": p1.astype(ml_dtypes.bfloat16),
    }
    in_maps = []
    for c in range(NCORES):
        m = dict(common)
        xr = np.roll(x, -ROWS * c, axis=0)[:XCOLS]
        xrT = np.ascontiguousarray(xr.T)  # [512, 1056]
        m["xT"] = np.concatenate(
            [xrT[k * D : (k + 1) * D] for k in range(4)], axis=1
        ).astype(ml_dtypes.bfloat16)
        in_maps.append(m)
    return in_maps


def _assemble(results):
    N = N_NODES
    O = np.zeros((N, N), np.float32)
    dd = np.arange(769)
    for c in range(NCORES):
        T = np.asarray(results[c]["out"][:, :769], np.float32)
        # rows grouped as (group, half, s) per GROUP_SIZES
        i_loc = np.empty(ROWS, np.int64)
        rbase = 0
        pbase = 0
        for gsz in GROUP_SIZES:
            sidx = np.arange(gsz)
            i_loc[rbase : rbase + gsz] = pbase + sidx
            i_loc[rbase + gsz : rbase + 2 * gsz] = pbase + sidx + 96
            rbase += 2 * gsz
            pbase += gsz
        gi = (ROWS * c + i_loc) % N
        cols = (gi[:, None] + dd[None, :]) % N
        O[gi[:, None], cols] = T
        O[cols, gi[:, None]] = T
    return O


def kernel(x, W_enc, b_enc, W1a, b1a, W1b, b1b, W2a, b2a, W2b, b2b):
    from concourse.bass_utils import run_bass_kernel_spmd

    global _compiled
    if _compiled is None:
        _compiled = _build_program()
    in_maps = _host_inputs(
        np.asarray(x, np.float32),
        np.asarray(W_enc, np.float32), np.asarray(b_enc, np.float32),
        np.asarray(W1a, np.float32), np.asarray(b1a, np.float32),
        np.asarray(W1b, np.float32), np.asarray(b1b, np.float32),
        np.asarray(W2a, np.float32), np.asarray(b2a, np.float32),
        np.asarray(W2b, np.float32), np.asarray(b2b, np.float32),
    )
    res = run_bass_kernel_spmd(_compiled, in_maps, list(range(NCORES)))
    return _assemble(res.results)


# revision 10
# speedup vs baseline: 1.0324x; 1.0040x over previous
"""Trainium2 Bass kernel v2 for nn_LCAMatrixModel (pairwise selu-MLP scoring).

o[i,j] = hardsigmoid( sum_h W2b[h]*selu(g[i,h]+g[j,h]+b2a[h]) + b2b ), symmetric.

Circulant band decomposition: each core owns 192 consecutive global rows
(core c: rows [192c, 192c+192)) of the band t[i, d] = o[i, (i+d) % N],
d in [0, 769) - exactly half the matrix. Inputs are host-rolled by 192c so
all cores run the same program on local rows [0, 192). Rows are processed
as 96 pairs (i, i+96) stacked on 128 partitions (2 x 64 h).

Per-pair math over the band window (u = g_i + g_j + b2a):
  sum_h w*selu(u) = lam*al*sum_h w*(rs + q) - lam*al*Sw,  with
  rs = relu(u)/al   and   q = e^{min(u,0)} = min(e^{g_i} * e^{g_j+b2a}, 1)
(the exp is SEPARABLE: E=e^g per node is precomputed, so the pairwise q is
just a multiply+min). o = clip(V/6 + 0.5, 0, 1). Every matmul uses the one
stationary pattern P1 = lam*al*w (g is stored pre-divided by al).

Engine balance via per-pair schemes (slot counts tunable):
  Z  : rs = DVE ts(add,max0);  q = DVE ts(mult,min1);  PE: P1@q + P1@rs
  Y2 : rs = ACT Relu(g2w+bias); q = DVE;               PE: P1@q + P1@rs
  W2 : rs = ACT; q = DVE; f = q+rs (DVE or Pool tt);   PE: P1@f  (half PE)
"""
import sys

sys.path.insert(0, "/opt/trn_rl_repo")

import numpy as np

N_NODES = 1536
RAW = 512
D = 128
H = 64
NCORES = 8
ROWS = N_NODES // NCORES          # 192 rows per core
PAIRS = ROWS // 2                 # 96 pairs (i, i+96)
GROUPS = 3                        # 3 groups of 32 pairs
BW = 772                          # padded band width (769 used)
GSPAN = 960                       # g2w width: max i (95) + BW + pad
XCOLS = 1056                      # g cols needed: GSPAN + 96 (bottom shift)
CW0, CW1 = 356, BW - 356          # psum chunk widths (356+416)
ECW = 352                         # encoder/prologue chunk width (3 x 352 = 1056)

LAM = 1.0507009873554805
AL = 1.6732632423543772

# per-group scheme quotas (fractions of 32): Z=DVE rs, Y2=ACT rs + 2mm,
# W2=ACT rs + f-combine + 1mm. Of W2, NWP_FR on Pool, rest DVE.
NZ_DEF = 15
NY_DEF = 3
NWP_DEF = 7
GROUP_SIZES = (32, 32, 32)
# per-group (NZ, NY, NW, NWP): front-load PE work, drain PE-light
GROUP_QUOTAS = None  # uniform per-group quotas

_compiled = None


def _build_program(NZ=NZ_DEF, NY=NY_DEF, NWP=NWP_DEF):
    import concourse.bacc as bacc
    import concourse.mybir as mybir
    import concourse.tile as tile

    F32 = mybir.dt.float32
    BF16 = mybir.dt.bfloat16
    AF = mybir.ActivationFunctionType
    OP = mybir.AluOpType

    NW = 32 - NZ - NY

    nc = bacc.Bacc("TRN2", target_bir_lowering=False, debug=False)

    # ---- DRAM I/O ----
    xT_d = nc.dram_tensor("xT", [D, 4 * XCOLS], BF16, kind="ExternalInput")
    cpack_d = nc.dram_tensor("cpack", [D, 6], F32, kind="ExternalInput")
    wab_d = nc.dram_tensor("wab", [D, 6 * H], BF16, kind="ExternalInput")
    w1bT_d = nc.dram_tensor("w1bT", [H, D], BF16, kind="ExternalInput")
    p1_d = nc.dram_tensor("p1", [D, 32 * H], BF16, kind="ExternalInput")
    out_d = nc.dram_tensor("out", [ROWS, BW], BF16, kind="ExternalOutput")

    with tile.TileContext(nc) as tc:
        with (
            tc.tile_pool(name="cst", bufs=1) as cst,
            tc.tile_pool(name="enc", bufs=1) as enc,
            tc.tile_pool(name="pre", bufs=3) as pre,
            tc.tile_pool(name="fq", bufs=28) as fq,
            tc.tile_pool(name="op", bufs=6) as opool,
            tc.tile_pool(name="ps", bufs=2, space="PSUM") as ps,
            tc.tile_pool(name="psp", bufs=4, space="PSUM") as psp,
        ):
            # ---- input DMAs (x first - encoder needs it asap) ----
            xt = enc.tile([D, 4 * XCOLS], BF16)
            nc.scalar.dma_start(xt[:, :], xT_d[:, :])
            cpack = cst.tile([D, 6], F32)
            nc.sync.dma_start(cpack[:], cpack_d[:])
            wab = cst.tile([D, 6 * H], BF16)
            nc.sync.dma_start(wab[:], wab_d[:])
            w1bT = cst.tile([H, D], BF16)
            nc.sync.dma_start(w1bT[:, :], w1bT_d[:])
            p1 = cst.tile([D, 32 * H], BF16)
            nc.sync.dma_start(p1[:], p1_d[:])
            b1b = cpack[:, 1:2]
            b2a2 = cpack[:, 2:3]
            bcmb = cpack[0:H, 3:4]
            kcol = cpack[0:H, 4:5]
            bcmbl = cpack[0:H, 0:1]   # lam * bcmb
            b1bl = cpack[:, 5:6]      # lam * b1b
            wcmb = wab[:, 0 : 4 * H]       # W1a@W_enc, 4 k-tiles of [128,64]
            w2aT = wab[:, 4 * H : 6 * H]
            NCH = XCOLS // ECW  # 3 chunks of 352

            # ---- selu helper: v = psum+b (bf16); negative branch via
            # min(lam*al*(e^v - 1), 0) so exp needs no pre-clamp; final
            # clamp+add fused into one scalar_tensor_tensor ----
            # ACT-direct variant (both branches from PSUM) - used for the
            # a1 layer, where ACT still has headroom early in the prologue
            def selu_act(out_ap, pa, b_raw, b_lam, p, nm):
                r = pre.tile([p, ECW], BF16, tag="selr", name=f"r_{nm}")
                nc.scalar.activation(r[:], pa, AF.Relu, bias=b_lam, scale=LAM)
                e = pre.tile([p, ECW], BF16, tag="sele", name=f"e_{nm}")
                nc.scalar.activation(e[:], pa, AF.Exp, bias=b_raw)
                t = pre.tile([p, ECW], BF16, tag="selt", name=f"t_{nm}")
                nc.vector.tensor_scalar(t[:], e[:], LAM * AL, -LAM * AL, OP.mult, OP.add)
                nc.vector.scalar_tensor_tensor(
                    out_ap, t[:], 0.0, r[:], OP.min, OP.add
                )

            def selu_from_psum(out_ap, pa, b_raw, b_lam, p, nm):
                v = pre.tile([p, ECW], BF16, tag="selv", name=f"v_{nm}")
                nc.vector.tensor_scalar(v[:], pa, b_raw, 0.0, OP.add, OP.add)
                r = pre.tile([p, ECW], BF16, tag="selr", name=f"r_{nm}")
                nc.vector.tensor_scalar(r[:], v[:], 0.0, LAM, OP.max, OP.mult)
                e = pre.tile([p, ECW], BF16, tag="sele", name=f"e_{nm}")
                nc.scalar.activation(e[:], v[:], AF.Exp)
                t = pre.tile([p, ECW], BF16, tag="selt", name=f"t_{nm}")
                nc.vector.tensor_scalar(t[:], e[:], LAM * AL, -LAM * AL, OP.mult, OP.add)
                nc.vector.scalar_tensor_tensor(
                    out_ap, t[:], 0.0, r[:], OP.min, OP.add
                )

            # ---- wavefront emission over (layer, chunk): keeps every
            # engine queue supplied while chains pipeline ----
            a1T = enc.tile([H, XCOLS], BF16)
            hT = enc.tile([D, XCOLS], BF16)
            g2dup = cst.tile([D, XCOLS], F32)
            g2w = cst.tile([D, GSPAN], BF16)
            E2w = cst.tile([D, GSPAN], BF16)
            gbs_raw = cst.tile([D, PAIRS], F32)
            gbs_s = cst.tile([D, PAIRS], F32)
            EIS = cst.tile([D, PAIRS], F32)

            def do_a1(c):
                # fused encoder+first layer: a1pre = x @ (W1a@W_enc).T
                sl = slice(c * ECW, (c + 1) * ECW)
                pa = psp.tile([H, ECW], F32, tag="pp", name=f"pa_{c}")
                for k in range(4):
                    nc.tensor.matmul(
                        pa[:],
                        wcmb[:, k * H : (k + 1) * H],
                        xt[:, k * XCOLS + c * ECW : k * XCOLS + (c + 1) * ECW],
                        start=(k == 0),
                        stop=(k == 3),
                    )
                selu_act(a1T[:, sl], pa[:], bcmb, bcmbl, H, f"a{c}")

            def do_h(c):
                sl = slice(c * ECW, (c + 1) * ECW)
                ph = psp.tile([D, ECW], F32, tag="pp", name=f"ph_{c}")
                nc.tensor.matmul(ph[:], w1bT[:, :], a1T[:, sl], start=True, stop=True)
                selu_from_psum(hT[:, sl], ph[:], b1b, b1bl, D, f"h{c}")

            def do_g(c):
                sl = slice(c * ECW, (c + 1) * ECW)
                pg = psp.tile([D, ECW], F32, tag="pp", name=f"pg_{c}")
                nc.tensor.matmul(pg[:], w2aT, hT[:, sl], start=True, stop=True)
                nc.scalar.activation(g2dup[:, sl], pg[:], AF.Copy)
                # build the g2w/E2w pieces this chunk enables (top: same cols,
                # bottom: cols shifted by -96), so the main loop can start
                # right after the last chunk
                t0, t1 = c * ECW, min((c + 1) * ECW, GSPAN)
                if t1 > t0:
                    nc.gpsimd.tensor_scalar_mul(
                        g2w[0:H, t0:t1], g2dup[0:H, t0:t1], 1.0 / AL
                    )
                    nc.scalar.activation(
                        E2w[0:H, t0:t1], g2dup[0:H, t0:t1], AF.Exp,
                        bias=b2a2[0:H, :],
                    )
                b0 = max(c * ECW - 96, 0)
                b1 = min((c + 1) * ECW - 96, GSPAN)
                if b1 > b0:
                    nc.gpsimd.tensor_scalar_mul(
                        g2w[H:D, b0:b1], g2dup[H:D, 96 + b0 : 96 + b1], 1.0 / AL
                    )
                    nc.scalar.activation(
                        E2w[H:D, b0:b1], g2dup[H:D, 96 + b0 : 96 + b1], AF.Exp,
                        bias=b2a2[H:D, :],
                    )
                if c == 0:
                    nc.gpsimd.tensor_copy(gbs_raw[0:H, :], g2dup[0:H, 0:PAIRS])
                    nc.gpsimd.tensor_copy(
                        gbs_raw[H:D, :], g2dup[H:D, 96 : 96 + PAIRS]
                    )
                    nc.vector.tensor_scalar(
                        gbs_s[:], gbs_raw[:], b2a2, 1.0 / AL, OP.add, OP.mult
                    )
                    nc.scalar.activation(EIS[:], gbs_raw[:], AF.Exp)

            layers = (do_a1, do_h, do_g)
            for step in range(len(layers) + NCH - 1):
                for c in range(NCH):
                    li = step - c
                    if 0 <= li < len(layers):
                        layers[li](c)

            # ================= main pairwise loop =================
            def finalize(G, pb, gsz, pt, c0, cw):
                # o = Relu(psum/6 + kb) ; out = min(o, 1) ; DMA out.
                # pair s of the group lands on psum rows s (top) and 32+s
                # (bottom). The min runs on Pool mid-stream but on DVE for
                # the last group (DVE is idle by then; Pool's slower op sits
                # on the kernel's critical tail).
                o = opool.tile([H, BW], BF16, tag="o", name=f"o_{G}_{c0}")
                nc.scalar.activation(
                    o[:, c0 : c0 + cw], pt[:], AF.Relu,
                    scale=1.0 / 6.0, bias=kcol,
                )
                oo = opool.tile([H, BW], BF16, tag="oo", name=f"oo_{G}_{c0}")
                if G == len(GROUP_SIZES) - 1:
                    nc.vector.tensor_scalar_min(
                        oo[:, c0 : c0 + cw], o[:, c0 : c0 + cw], 1.0
                    )
                else:
                    nc.gpsimd.tensor_scalar_min(
                        oo[:, c0 : c0 + cw], o[:, c0 : c0 + cw], 1.0
                    )
                if gsz == 32:
                    # both row halves are contiguous: one descriptor
                    nc.sync.dma_start(
                        out_d[2 * pb : 2 * pb + 64, c0 : c0 + cw],
                        oo[:, c0 : c0 + cw],
                    )
                else:
                    nc.sync.dma_start(
                        out_d[2 * pb : 2 * pb + gsz, c0 : c0 + cw],
                        oo[0:gsz, c0 : c0 + cw],
                    )
                    nc.sync.dma_start(
                        out_d[2 * pb + gsz : 2 * pb + 2 * gsz, c0 : c0 + cw],
                        oo[32 : 32 + gsz, c0 : c0 + cw],
                    )

            pending_fin = []
            pbase = 0

            for G, GSZ in enumerate(GROUP_SIZES):
                pc0 = ps.tile([H, CW0], F32, tag="pc0", name=f"pc0_{G}")
                pc1 = ps.tile([H, CW1], F32, tag="pcx", name=f"pc1_{G}")
                started0 = False
                started1 = [False]

                # proportionally interleaved scheme schedule (Bresenham)
                if GROUP_QUOTAS is not None:
                    NZg, NYg, NWg, NWPg = GROUP_QUOTAS[G]
                else:
                    NZg, NYg, NWg, NWPg = NZ, NY, NW, NWP
                quota = {"Z": NZg * GSZ / 32.0, "Y2": NYg * GSZ / 32.0,
                         "W2": NWg * GSZ / 32.0}
                counts = {k: 0 for k in quota}
                sched = []
                for s in range(GSZ):
                    k = max(quota, key=lambda k: quota[k] * (s + 1) / GSZ - counts[k])
                    sched.append(k)
                    counts[k] += 1
                NWG = counts["W2"]
                NWPG = min(NWPg, NWG)
                w2_idx = 0
                # last slot must not be W2 (deferred W2 work flushes before it
                # so the group's stop-flag matmul stays last)
                if sched[GSZ - 1] == "W2":
                    for si in range(GSZ - 2, -1, -1):
                        if sched[si] != "W2":
                            sched[GSZ - 1], sched[si] = sched[si], sched[GSZ - 1]
                            break
                deferred = []

                for s in range(GSZ):
                    p = pbase + s
                    scheme = sched[s]
                    # q = min(Ei * E2j, 1) = e^{min(u,0)}   (always DVE, 4x)
                    q = fq.tile([D, BW], BF16, tag="q", name=f"q_{G}_{s}")
                    nc.vector.tensor_scalar(
                        q[:], E2w[:, p : p + BW], EIS[:, p : p + 1],
                        1.0, OP.mult, OP.min,
                    )
                    # rs = relu(u)/al : DVE for Z slots, ACT otherwise
                    rs = fq.tile([D, BW], BF16, tag="rs", name=f"rs_{G}_{s}")
                    if scheme == "Z":
                        nc.vector.tensor_scalar(
                            rs[:], g2w[:, p : p + BW], gbs_s[:, p : p + 1],
                            0.0, OP.add, OP.max,
                        )
                    else:
                        nc.scalar.activation(
                            rs[:], g2w[:, p : p + BW], AF.Relu,
                            bias=gbs_s[:, p : p + 1],
                        )
                    def emit_mms(slot, movers, last):
                        nonlocal started0
                        pat1 = p1[:, H * slot : H * (slot + 1)]
                        for pt, c0, cw in ((pc0, 0, CW0), (pc1, CW0, CW1)):
                            for mi, mv in enumerate(movers):
                                nc.tensor.matmul(
                                    pt[:], pat1, mv[:, c0 : c0 + cw],
                                    start=(not started0 and c0 == 0 and mi == 0)
                                    or (not started1[0] and c0 == CW0 and mi == 0),
                                    stop=last and mi == len(movers) - 1,
                                    skip_group_check=True,
                                )
                                if c0 == CW0:
                                    started1[0] = True
                        started0 = True

                    if s == GSZ - 1:
                        # flush deferred W2 work so this slot's mms close psum
                        for emit in deferred:
                            emit()
                        deferred = []

                    if scheme == "W2":
                        f = fq.tile([D, BW], BF16, tag="f", name=f"f_{G}_{s}")
                        on_pool = NWPG > 0 and (
                            (w2_idx + 1) * NWPG // NWG > w2_idx * NWPG // NWG
                        )
                        w2_idx += 1

                        def emit_w2(f=f, q=q, rs=rs, slot=s, on_pool=on_pool):
                            if on_pool:
                                nc.gpsimd.tensor_tensor(f[:], q[:], rs[:], OP.add)
                            else:
                                nc.vector.tensor_tensor(f[:], q[:], rs[:], OP.add)
                            emit_mms(slot, (f,), False)

                        deferred.append(emit_w2)
                    else:
                        emit_mms(s, (q, rs), s == GSZ - 1)
                        # drain one deferred W2 behind this slot's ready work
                        if len(deferred) > 1:
                            deferred.pop(0)()
                    # emit previous group's finalize mid-stream so it doesn't
                    # head-of-line-block this group's producer ops
                    if s == 6 and pending_fin:
                        for fin in pending_fin:
                            finalize(*fin)
                        pending_fin = []
                pending_fin.append((G, pbase, GSZ, pc0, 0, CW0))
                pending_fin.append((G, pbase, GSZ, pc1, CW0, CW1))
                pbase += GSZ
            for fin in pending_fin:
                finalize(*fin)

    nc.compile()
    return nc


def _host_inputs(x, W_enc, b_enc, W1a, b1a, W1b, b1b, W2a, b2a, W2b, b2b):
    w = W2b[0].astype(np.float64)
    SW = float(w.sum())
    KB = (-LAM * AL * SW + float(b2b[0])) / 6.0 + 0.5

    import ml_dtypes

    lam_al_w = (LAM * AL * w).astype(np.float32)
    p1 = np.zeros((D, 32 * H), np.float32)
    for s in range(32):
        p1[0:H, H * s + s] = lam_al_w
        p1[H:D, H * s + 32 + s] = lam_al_w

    W_combo = (W1a.astype(np.float64) @ W_enc.astype(np.float64)).astype(
        np.float32
    )  # [H, RAW]
    b_combo = (W1a.astype(np.float64) @ b_enc.astype(np.float64)
               + b1a.astype(np.float64)).astype(np.float32)
    cpack = np.zeros((D, 6), np.float32)
    cpack[0:H, 0] = LAM * b_combo
    cpack[:, 1] = b1b
    cpack[:, 2] = np.concatenate([b2a, b2a])
    cpack[0:H, 3] = b_combo
    cpack[0:H, 4] = KB
    cpack[:, 5] = LAM * b1b
    w2aT = np.ascontiguousarray(W2a.T)
    wcmbT = np.concatenate(
        [W_combo.T[k * D : (k + 1) * D] for k in range(4)], axis=1
    )  # [128, 4*64]
    wab = np.concatenate([wcmbT, w2aT, w2aT], axis=1)
    common = {
        "cpack": cpack,
        "wab": wab.astype(ml_dtypes.bfloat16),
        "w1bT": np.ascontiguousarray(W1b.T).astype(ml_dtypes.bfloat16),
        "p1": p1.astype(ml_dtypes.bfloat16),
    }
    in_maps = []
    for c in range(NCORES):
        m = dict(common)
        xr = np.roll(x, -ROWS * c, axis=0)[:XCOLS]
        xrT = np.ascontiguousarray(xr.T)  # [512, 1056]
        m["xT"] = np.concatenate(
            [xrT[k * D : (k + 1) * D] for k in range(4)], axis=1
        ).astype(ml_dtypes.bfloat16)
        in_maps.append(m)
    return in_maps


def _assemble(results):
    N = N_NODES
    O = np.zeros((N, N), np.float32)
    dd = np.arange(769)
    for c in range(NCORES):
        T = np.asarray(results[c]["out"][:, :769], np.float32)
        # rows grouped as (group, half, s) per GROUP_SIZES
        i_loc = np.empty(ROWS, np.int64)
        rbase = 0
        pbase = 0
        for gsz in GROUP_SIZES:
            sidx = np.arange(gsz)
            i_loc[rbase : rbase + gsz] = pbase + sidx
            i_loc[rbase + gsz : rbase + 2 * gsz] = pbase + sidx + 96
            rbase += 2 * gsz
            pbase += gsz
        gi = (ROWS * c + i_loc) % N
        cols = (gi[:, None] + dd[None, :]) % N
        O[gi[:, None], cols] = T
        O[cols, gi[:, None]] = T
    return O


def kernel(x, W_enc, b_enc, W1a, b1a, W1b, b1b, W2a, b2a, W2b, b2b):
    from concourse.bass_utils import run_bass_kernel_spmd

    global _compiled
    if _compiled is None:
        _compiled = _build_program()
    in_maps = _host_inputs(
        np.asarray(x, np.float32),
        np.asarray(W_enc, np.float32), np.asarray(b_enc, np.float32),
        np.asarray(W1a, np.float32), np.asarray(b1a, np.float32),
        np.asarray(W1b, np.float32), np.asarray(b1b, np.float32),
        np.asarray(W2a, np.float32), np.asarray(b2a, np.float32),
        np.asarray(W2b, np.float32), np.asarray(b2b, np.float32),
    )
    res = run_bass_kernel_spmd(_compiled, in_maps, list(range(NCORES)))
    return _assemble(res.results)


# revision 11
# speedup vs baseline: 1.0493x; 1.0164x over previous
"""Trainium2 Bass kernel v2 for nn_LCAMatrixModel (pairwise selu-MLP scoring).

o[i,j] = hardsigmoid( sum_h W2b[h]*selu(g[i,h]+g[j,h]+b2a[h]) + b2b ), symmetric.

Circulant band decomposition: each core owns 192 consecutive global rows
(core c: rows [192c, 192c+192)) of the band t[i, d] = o[i, (i+d) % N],
d in [0, 769) - exactly half the matrix. Inputs are host-rolled by 192c so
all cores run the same program on local rows [0, 192). Rows are processed
as 96 pairs (i, i+96) stacked on 128 partitions (2 x 64 h).

Per-pair math over the band window (u = g_i + g_j + b2a):
  sum_h w*selu(u) = lam*al*sum_h w*(rs + q) - lam*al*Sw,  with
  rs = relu(u)/al   and   q = e^{min(u,0)} = min(e^{g_i} * e^{g_j+b2a}, 1)
(the exp is SEPARABLE: E=e^g per node is precomputed, so the pairwise q is
just a multiply+min). o = clip(V/6 + 0.5, 0, 1). Every matmul uses the one
stationary pattern P1 = lam*al*w (g is stored pre-divided by al).

Engine balance via per-pair schemes (slot counts tunable):
  Z  : rs = DVE ts(add,max0);  q = DVE ts(mult,min1);  PE: P1@q + P1@rs
  Y2 : rs = ACT Relu(g2w+bias); q = DVE;               PE: P1@q + P1@rs
  W2 : rs = ACT; q = DVE; f = q+rs (DVE or Pool tt);   PE: P1@f  (half PE)
"""
import sys

sys.path.insert(0, "/opt/trn_rl_repo")

import numpy as np

N_NODES = 1536
RAW = 512
D = 128
H = 64
NCORES = 8
ROWS = N_NODES // NCORES          # 192 rows per core
PAIRS = ROWS // 2                 # 96 pairs (i, i+96)
GROUPS = 3                        # 3 groups of 32 pairs
BW = 769                          # band width
GSPAN = 868                       # g2w width: max i (95) + BW + 1
XCOLS = 966                       # g cols needed: >= 963 (bottom shift+span)
CW0, CW1 = 356, BW - 356          # psum chunk widths (356+413)
ECW = 322                         # encoder/prologue chunk width (3 x 322 = 966)

LAM = 1.0507009873554805
AL = 1.6732632423543772

# per-group scheme quotas (fractions of 32): Z=DVE rs, Y2=ACT rs + 2mm,
# W2=ACT rs + f-combine + 1mm. Of W2, NWP_FR on Pool, rest DVE.
NZ_DEF = 15
NY_DEF = 3
NWP_DEF = 7
GROUP_SIZES = (32, 32, 32)
# per-group (NZ, NY, NW, NWP): front-load PE work, drain PE-light
GROUP_QUOTAS = None  # uniform per-group quotas

_compiled = None


def _build_program(NZ=NZ_DEF, NY=NY_DEF, NWP=NWP_DEF):
    import concourse.bacc as bacc
    import concourse.mybir as mybir
    import concourse.tile as tile

    F32 = mybir.dt.float32
    BF16 = mybir.dt.bfloat16
    AF = mybir.ActivationFunctionType
    OP = mybir.AluOpType

    NW = 32 - NZ - NY

    nc = bacc.Bacc("TRN2", target_bir_lowering=False, debug=False)

    # ---- DRAM I/O ----
    xT_d = nc.dram_tensor("xT", [D, 4 * XCOLS], BF16, kind="ExternalInput")
    cpack_d = nc.dram_tensor("cpack", [D, 6], F32, kind="ExternalInput")
    wab_d = nc.dram_tensor("wab", [D, 6 * H], BF16, kind="ExternalInput")
    w1bT_d = nc.dram_tensor("w1bT", [H, D], BF16, kind="ExternalInput")
    p1_d = nc.dram_tensor("p1", [D, 32 * H], BF16, kind="ExternalInput")
    out_d = nc.dram_tensor("out", [ROWS, BW], BF16, kind="ExternalOutput")

    with tile.TileContext(nc) as tc:
        with (
            tc.tile_pool(name="cst", bufs=1) as cst,
            tc.tile_pool(name="enc", bufs=1) as enc,
            tc.tile_pool(name="pre", bufs=3) as pre,
            tc.tile_pool(name="fq", bufs=28) as fq,
            tc.tile_pool(name="op", bufs=6) as opool,
            tc.tile_pool(name="ps", bufs=2, space="PSUM") as ps,
            tc.tile_pool(name="psp", bufs=4, space="PSUM") as psp,
        ):
            # ---- input DMAs (x first - encoder needs it asap) ----
            xt = enc.tile([D, 4 * XCOLS], BF16)
            nc.scalar.dma_start(xt[:, :], xT_d[:, :])
            cpack = cst.tile([D, 6], F32)
            nc.sync.dma_start(cpack[:], cpack_d[:])
            wab = cst.tile([D, 6 * H], BF16)
            nc.sync.dma_start(wab[:], wab_d[:])
            w1bT = cst.tile([H, D], BF16)
            nc.sync.dma_start(w1bT[:, :], w1bT_d[:])
            p1 = cst.tile([D, 32 * H], BF16)
            nc.sync.dma_start(p1[:], p1_d[:])
            b1b = cpack[:, 1:2]
            b2a2 = cpack[:, 2:3]
            bcmb = cpack[0:H, 3:4]
            kcol = cpack[0:H, 4:5]
            bcmbl = cpack[0:H, 0:1]   # lam * bcmb
            b1bl = cpack[:, 5:6]      # lam * b1b
            wcmb = wab[:, 0 : 4 * H]       # W1a@W_enc, 4 k-tiles of [128,64]
            w2aT = wab[:, 4 * H : 6 * H]
            NCH = XCOLS // ECW  # 3 chunks of 352

            # ---- selu helper: v = psum+b (bf16); negative branch via
            # min(lam*al*(e^v - 1), 0) so exp needs no pre-clamp; final
            # clamp+add fused into one scalar_tensor_tensor ----
            # ACT-direct variant (both branches from PSUM) - used for the
            # a1 layer, where ACT still has headroom early in the prologue
            def selu_act(out_ap, pa, b_raw, b_lam, p, nm):
                r = pre.tile([p, ECW], BF16, tag="selr", name=f"r_{nm}")
                nc.scalar.activation(r[:], pa, AF.Relu, bias=b_lam, scale=LAM)
                e = pre.tile([p, ECW], BF16, tag="sele", name=f"e_{nm}")
                nc.scalar.activation(e[:], pa, AF.Exp, bias=b_raw)
                t = pre.tile([p, ECW], BF16, tag="selt", name=f"t_{nm}")
                nc.vector.tensor_scalar(t[:], e[:], LAM * AL, -LAM * AL, OP.mult, OP.add)
                nc.vector.scalar_tensor_tensor(
                    out_ap, t[:], 0.0, r[:], OP.min, OP.add
                )

            def selu_from_psum(out_ap, pa, b_raw, b_lam, p, nm):
                v = pre.tile([p, ECW], BF16, tag="selv", name=f"v_{nm}")
                nc.vector.tensor_scalar(v[:], pa, b_raw, 0.0, OP.add, OP.add)
                r = pre.tile([p, ECW], BF16, tag="selr", name=f"r_{nm}")
                nc.vector.tensor_scalar(r[:], v[:], 0.0, LAM, OP.max, OP.mult)
                e = pre.tile([p, ECW], BF16, tag="sele", name=f"e_{nm}")
                nc.scalar.activation(e[:], v[:], AF.Exp)
                t = pre.tile([p, ECW], BF16, tag="selt", name=f"t_{nm}")
                nc.vector.tensor_scalar(t[:], e[:], LAM * AL, -LAM * AL, OP.mult, OP.add)
                nc.vector.scalar_tensor_tensor(
                    out_ap, t[:], 0.0, r[:], OP.min, OP.add
                )

            # ---- wavefront emission over (layer, chunk): keeps every
            # engine queue supplied while chains pipeline ----
            a1T = enc.tile([H, XCOLS], BF16)
            hT = enc.tile([D, XCOLS], BF16)
            g2dup = cst.tile([D, XCOLS], F32)
            g2w = cst.tile([D, GSPAN], BF16)
            E2w = cst.tile([D, GSPAN], BF16)
            gbs_raw = cst.tile([D, PAIRS], F32)
            gbs_s = cst.tile([D, PAIRS], F32)
            EIS = cst.tile([D, PAIRS], F32)

            def do_a1(c):
                # fused encoder+first layer: a1pre = x @ (W1a@W_enc).T
                sl = slice(c * ECW, (c + 1) * ECW)
                pa = psp.tile([H, ECW], F32, tag="pp", name=f"pa_{c}")
                for k in range(4):
                    nc.tensor.matmul(
                        pa[:],
                        wcmb[:, k * H : (k + 1) * H],
                        xt[:, k * XCOLS + c * ECW : k * XCOLS + (c + 1) * ECW],
                        start=(k == 0),
                        stop=(k == 3),
                    )
                selu_act(a1T[:, sl], pa[:], bcmb, bcmbl, H, f"a{c}")

            def do_h(c):
                sl = slice(c * ECW, (c + 1) * ECW)
                ph = psp.tile([D, ECW], F32, tag="pp", name=f"ph_{c}")
                nc.tensor.matmul(ph[:], w1bT[:, :], a1T[:, sl], start=True, stop=True)
                selu_from_psum(hT[:, sl], ph[:], b1b, b1bl, D, f"h{c}")

            def do_g(c):
                sl = slice(c * ECW, (c + 1) * ECW)
                pg = psp.tile([D, ECW], F32, tag="pp", name=f"pg_{c}")
                nc.tensor.matmul(pg[:], w2aT, hT[:, sl], start=True, stop=True)
                nc.scalar.activation(g2dup[:, sl], pg[:], AF.Copy)
                # build the g2w/E2w pieces this chunk enables (top: same cols,
                # bottom: cols shifted by -96), so the main loop can start
                # right after the last chunk
                t0, t1 = c * ECW, min((c + 1) * ECW, GSPAN)
                if t1 > t0:
                    nc.gpsimd.tensor_scalar_mul(
                        g2w[0:H, t0:t1], g2dup[0:H, t0:t1], 1.0 / AL
                    )
                    nc.scalar.activation(
                        E2w[0:H, t0:t1], g2dup[0:H, t0:t1], AF.Exp,
                        bias=b2a2[0:H, :],
                    )
                b0 = max(c * ECW - 96, 0)
                b1 = min((c + 1) * ECW - 96, GSPAN)
                if b1 > b0:
                    nc.gpsimd.tensor_scalar_mul(
                        g2w[H:D, b0:b1], g2dup[H:D, 96 + b0 : 96 + b1], 1.0 / AL
                    )
                    nc.scalar.activation(
                        E2w[H:D, b0:b1], g2dup[H:D, 96 + b0 : 96 + b1], AF.Exp,
                        bias=b2a2[H:D, :],
                    )
                if c == 0:
                    nc.gpsimd.tensor_copy(gbs_raw[0:H, :], g2dup[0:H, 0:PAIRS])
                    nc.gpsimd.tensor_copy(
                        gbs_raw[H:D, :], g2dup[H:D, 96 : 96 + PAIRS]
                    )
                    nc.vector.tensor_scalar(
                        gbs_s[:], gbs_raw[:], b2a2, 1.0 / AL, OP.add, OP.mult
                    )
                    nc.scalar.activation(EIS[:], gbs_raw[:], AF.Exp)

            layers = (do_a1, do_h, do_g)
            for step in range(len(layers) + NCH - 1):
                for c in range(NCH):
                    li = step - c
                    if 0 <= li < len(layers):
                        layers[li](c)

            # ================= main pairwise loop =================
            def finalize(G, pb, gsz, pt, c0, cw):
                # o = Relu(psum/6 + kb) ; out = min(o, 1) ; DMA out.
                # pair s of the group lands on psum rows s (top) and 32+s
                # (bottom). The min runs on Pool mid-stream but on DVE for
                # the last group (DVE is idle by then; Pool's slower op sits
                # on the kernel's critical tail).
                o = opool.tile([H, BW], BF16, tag="o", name=f"o_{G}_{c0}")
                nc.scalar.activation(
                    o[:, c0 : c0 + cw], pt[:], AF.Relu,
                    scale=1.0 / 6.0, bias=kcol,
                )
                oo = opool.tile([H, BW], BF16, tag="oo", name=f"oo_{G}_{c0}")
                if G == len(GROUP_SIZES) - 1:
                    nc.vector.tensor_scalar_min(
                        oo[:, c0 : c0 + cw], o[:, c0 : c0 + cw], 1.0
                    )
                else:
                    nc.gpsimd.tensor_scalar_min(
                        oo[:, c0 : c0 + cw], o[:, c0 : c0 + cw], 1.0
                    )
                if gsz == 32:
                    # both row halves are contiguous: one descriptor
                    nc.sync.dma_start(
                        out_d[2 * pb : 2 * pb + 64, c0 : c0 + cw],
                        oo[:, c0 : c0 + cw],
                    )
                else:
                    nc.sync.dma_start(
                        out_d[2 * pb : 2 * pb + gsz, c0 : c0 + cw],
                        oo[0:gsz, c0 : c0 + cw],
                    )
                    nc.sync.dma_start(
                        out_d[2 * pb + gsz : 2 * pb + 2 * gsz, c0 : c0 + cw],
                        oo[32 : 32 + gsz, c0 : c0 + cw],
                    )

            pending_fin = []
            pbase = 0

            for G, GSZ in enumerate(GROUP_SIZES):
                pc0 = ps.tile([H, CW0], F32, tag="pc0", name=f"pc0_{G}")
                pc1 = ps.tile([H, CW1], F32, tag="pcx", name=f"pc1_{G}")
                started0 = False
                started1 = [False]

                # proportionally interleaved scheme schedule (Bresenham)
                if GROUP_QUOTAS is not None:
                    NZg, NYg, NWg, NWPg = GROUP_QUOTAS[G]
                else:
                    NZg, NYg, NWg, NWPg = NZ, NY, NW, NWP
                quota = {"Z": NZg * GSZ / 32.0, "Y2": NYg * GSZ / 32.0,
                         "W2": NWg * GSZ / 32.0}
                counts = {k: 0 for k in quota}
                sched = []
                for s in range(GSZ):
                    k = max(quota, key=lambda k: quota[k] * (s + 1) / GSZ - counts[k])
                    sched.append(k)
                    counts[k] += 1
                NWG = counts["W2"]
                NWPG = min(NWPg, NWG)
                w2_idx = 0
                # last slot must not be W2 (deferred W2 work flushes before it
                # so the group's stop-flag matmul stays last)
                if sched[GSZ - 1] == "W2":
                    for si in range(GSZ - 2, -1, -1):
                        if sched[si] != "W2":
                            sched[GSZ - 1], sched[si] = sched[si], sched[GSZ - 1]
                            break
                deferred = []

                for s in range(GSZ):
                    p = pbase + s
                    scheme = sched[s]
                    # q = min(Ei * E2j, 1) = e^{min(u,0)}   (always DVE, 4x)
                    q = fq.tile([D, BW], BF16, tag="q", name=f"q_{G}_{s}")
                    nc.vector.tensor_scalar(
                        q[:], E2w[:, p : p + BW], EIS[:, p : p + 1],
                        1.0, OP.mult, OP.min,
                    )
                    # rs = relu(u)/al : DVE for Z slots, ACT otherwise
                    rs = fq.tile([D, BW], BF16, tag="rs", name=f"rs_{G}_{s}")
                    if scheme == "Z":
                        nc.vector.tensor_scalar(
                            rs[:], g2w[:, p : p + BW], gbs_s[:, p : p + 1],
                            0.0, OP.add, OP.max,
                        )
                    else:
                        nc.scalar.activation(
                            rs[:], g2w[:, p : p + BW], AF.Relu,
                            bias=gbs_s[:, p : p + 1],
                        )
                    def emit_mms(slot, movers, last):
                        nonlocal started0
                        pat1 = p1[:, H * slot : H * (slot + 1)]
                        for pt, c0, cw in ((pc0, 0, CW0), (pc1, CW0, CW1)):
                            for mi, mv in enumerate(movers):
                                nc.tensor.matmul(
                                    pt[:], pat1, mv[:, c0 : c0 + cw],
                                    start=(not started0 and c0 == 0 and mi == 0)
                                    or (not started1[0] and c0 == CW0 and mi == 0),
                                    stop=last and mi == len(movers) - 1,
                                    skip_group_check=True,
                                )
                                if c0 == CW0:
                                    started1[0] = True
                        started0 = True

                    if s == GSZ - 1:
                        # flush deferred W2 work so this slot's mms close psum
                        for emit in deferred:
                            emit()
                        deferred = []

                    if scheme == "W2":
                        f = fq.tile([D, BW], BF16, tag="f", name=f"f_{G}_{s}")
                        on_pool = NWPG > 0 and (
                            (w2_idx + 1) * NWPG // NWG > w2_idx * NWPG // NWG
                        )
                        w2_idx += 1

                        def emit_w2(f=f, q=q, rs=rs, slot=s, on_pool=on_pool):
                            if on_pool:
                                nc.gpsimd.tensor_tensor(f[:], q[:], rs[:], OP.add)
                            else:
                                nc.vector.tensor_tensor(f[:], q[:], rs[:], OP.add)
                            emit_mms(slot, (f,), False)

                        deferred.append(emit_w2)
                    else:
                        emit_mms(s, (q, rs), s == GSZ - 1)
                        # drain one deferred W2 behind this slot's ready work
                        if len(deferred) > 1:
                            deferred.pop(0)()
                    # emit previous group's finalize mid-stream so it doesn't
                    # head-of-line-block this group's producer ops
                    if s == 6 and pending_fin:
                        for fin in pending_fin:
                            finalize(*fin)
                        pending_fin = []
                pending_fin.append((G, pbase, GSZ, pc0, 0, CW0))
                pending_fin.append((G, pbase, GSZ, pc1, CW0, CW1))
                pbase += GSZ
            for fin in pending_fin:
                finalize(*fin)

    nc.compile()
    return nc


def _host_inputs(x, W_enc, b_enc, W1a, b1a, W1b, b1b, W2a, b2a, W2b, b2b):
    w = W2b[0].astype(np.float64)
    SW = float(w.sum())
    KB = (-LAM * AL * SW + float(b2b[0])) / 6.0 + 0.5

    import ml_dtypes

    lam_al_w = (LAM * AL * w).astype(np.float32)
    p1 = np.zeros((D, 32 * H), np.float32)
    for s in range(32):
        p1[0:H, H * s + s] = lam_al_w
        p1[H:D, H * s + 32 + s] = lam_al_w

    W_combo = (W1a.astype(np.float64) @ W_enc.astype(np.float64)).astype(
        np.float32
    )  # [H, RAW]
    b_combo = (W1a.astype(np.float64) @ b_enc.astype(np.float64)
               + b1a.astype(np.float64)).astype(np.float32)
    cpack = np.zeros((D, 6), np.float32)
    cpack[0:H, 0] = LAM * b_combo
    cpack[:, 1] = b1b
    cpack[:, 2] = np.concatenate([b2a, b2a])
    cpack[0:H, 3] = b_combo
    cpack[0:H, 4] = KB
    cpack[:, 5] = LAM * b1b
    w2aT = np.ascontiguousarray(W2a.T)
    wcmbT = np.concatenate(
        [W_combo.T[k * D : (k + 1) * D] for k in range(4)], axis=1
    )  # [128, 4*64]
    wab = np.concatenate([wcmbT, w2aT, w2aT], axis=1)
    common = {
        "cpack": cpack,
        "wab": wab.astype(ml_dtypes.bfloat16),
        "w1bT": np.ascontiguousarray(W1b.T).astype(ml_dtypes.bfloat16),
        "p1": p1.astype(ml_dtypes.bfloat16),
    }
    in_maps = []
    for c in range(NCORES):
        m = dict(common)
        xr = np.roll(x, -ROWS * c, axis=0)[:XCOLS]
        xrT = np.ascontiguousarray(xr.T)  # [512, 1056]
        m["xT"] = np.concatenate(
            [xrT[k * D : (k + 1) * D] for k in range(4)], axis=1
        ).astype(ml_dtypes.bfloat16)
        in_maps.append(m)
    return in_maps


def _assemble(results):
    N = N_NODES
    O = np.zeros((N, N), np.float32)
    dd = np.arange(769)
    for c in range(NCORES):
        T = np.asarray(results[c]["out"][:, :769], np.float32)
        # rows grouped as (group, half, s) per GROUP_SIZES
        i_loc = np.empty(ROWS, np.int64)
        rbase = 0
        pbase = 0
        for gsz in GROUP_SIZES:
            sidx = np.arange(gsz)
            i_loc[rbase : rbase + gsz] = pbase + sidx
            i_loc[rbase + gsz : rbase + 2 * gsz] = pbase + sidx + 96
            rbase += 2 * gsz
            pbase += gsz
        gi = (ROWS * c + i_loc) % N
        cols = (gi[:, None] + dd[None, :]) % N
        O[gi[:, None], cols] = T
        O[cols, gi[:, None]] = T
    return O


def kernel(x, W_enc, b_enc, W1a, b1a, W1b, b1b, W2a, b2a, W2b, b2b):
    from concourse.bass_utils import run_bass_kernel_spmd

    global _compiled
    if _compiled is None:
        _compiled = _build_program()
    in_maps = _host_inputs(
        np.asarray(x, np.float32),
        np.asarray(W_enc, np.float32), np.asarray(b_enc, np.float32),
        np.asarray(W1a, np.float32), np.asarray(b1a, np.float32),
        np.asarray(W1b, np.float32), np.asarray(b1b, np.float32),
        np.asarray(W2a, np.float32), np.asarray(b2a, np.float32),
        np.asarray(W2b, np.float32), np.asarray(b2b, np.float32),
    )
    res = run_bass_kernel_spmd(_compiled, in_maps, list(range(NCORES)))
    return _assemble(res.results)


# revision 12
# speedup vs baseline: 1.0597x; 1.0100x over previous
"""Trainium2 Bass kernel v2 for nn_LCAMatrixModel (pairwise selu-MLP scoring).

o[i,j] = hardsigmoid( sum_h W2b[h]*selu(g[i,h]+g[j,h]+b2a[h]) + b2b ), symmetric.

Circulant band decomposition: each core owns 192 consecutive global rows
(core c: rows [192c, 192c+192)) of the band t[i, d] = o[i, (i+d) % N],
d in [0, 769) - exactly half the matrix. Inputs are host-rolled by 192c so
all cores run the same program on local rows [0, 192). Rows are processed
as 96 pairs (i, i+96) stacked on 128 partitions (2 x 64 h).

Per-pair math over the band window (u = g_i + g_j + b2a):
  sum_h w*selu(u) = lam*al*sum_h w*(rs + q) - lam*al*Sw,  with
  rs = relu(u)/al   and   q = e^{min(u,0)} = min(e^{g_i} * e^{g_j+b2a}, 1)
(the exp is SEPARABLE: E=e^g per node is precomputed, so the pairwise q is
just a multiply+min). o = clip(V/6 + 0.5, 0, 1). Every matmul uses the one
stationary pattern P1 = lam*al*w (g is stored pre-divided by al).

Engine balance via per-pair schemes (slot counts tunable):
  Z  : rs = DVE ts(add,max0);  q = DVE ts(mult,min1);  PE: P1@q + P1@rs
  Y2 : rs = ACT Relu(g2w+bias); q = DVE;               PE: P1@q + P1@rs
  W2 : rs = ACT; q = DVE; f = q+rs (DVE or Pool tt);   PE: P1@f  (half PE)
"""
import sys

sys.path.insert(0, "/opt/trn_rl_repo")

import numpy as np

N_NODES = 1536
RAW = 512
D = 128
H = 64
NCORES = 8
ROWS = N_NODES // NCORES          # 192 rows per core
PAIRS = ROWS // 2                 # 96 pairs (i, i+96)
GROUPS = 3                        # 3 groups of 32 pairs
BW = 769                          # band width
GSPAN = 868                       # g2w width: max i (95) + BW + 1
XCOLS = 966                       # g cols needed: >= 963 (bottom shift+span)
CW0, CW1 = 356, BW - 356          # psum chunk widths (356+413)
ECW = 322                         # encoder/prologue chunk width (3 x 322 = 966)

LAM = 1.0507009873554805
AL = 1.6732632423543772

# per-group scheme quotas (fractions of 32): Z=DVE rs, Y2=ACT rs + 2mm,
# W2=ACT rs + f-combine + 1mm. Of W2, NWP_FR on Pool, rest DVE.
NZ_DEF = 15
NY_DEF = 3
NWP_DEF = 7
GROUP_SIZES = (32, 32, 32)
# per-group (NZ, NY, NW, NWP): front-load PE work, drain PE-light
GROUP_QUOTAS = None  # uniform per-group quotas

_compiled = None


def _build_program(NZ=NZ_DEF, NY=NY_DEF, NWP=NWP_DEF):
    import concourse.bacc as bacc
    import concourse.mybir as mybir
    import concourse.tile as tile

    F32 = mybir.dt.float32
    BF16 = mybir.dt.bfloat16
    AF = mybir.ActivationFunctionType
    OP = mybir.AluOpType

    NW = 32 - NZ - NY

    nc = bacc.Bacc("TRN2", target_bir_lowering=False, debug=False)

    # ---- DRAM I/O ----
    xT_d = nc.dram_tensor("xT", [D, 4 * XCOLS], BF16, kind="ExternalInput")
    cpack_d = nc.dram_tensor("cpack", [D, 6], F32, kind="ExternalInput")
    wab_d = nc.dram_tensor("wab", [D, 6 * H], BF16, kind="ExternalInput")
    w1bT_d = nc.dram_tensor("w1bT", [H, D], BF16, kind="ExternalInput")
    p1_d = nc.dram_tensor("p1", [D, 32 * H], BF16, kind="ExternalInput")
    out_d = nc.dram_tensor("out", [ROWS, BW], BF16, kind="ExternalOutput")

    with tile.TileContext(nc) as tc:
        with (
            tc.tile_pool(name="cst", bufs=1) as cst,
            tc.tile_pool(name="enc", bufs=1) as enc,
            tc.tile_pool(name="pre", bufs=3) as pre,
            tc.tile_pool(name="fq", bufs=28) as fq,
            tc.tile_pool(name="op", bufs=6) as opool,
            tc.tile_pool(name="ps", bufs=2, space="PSUM") as ps,
            tc.tile_pool(name="psp", bufs=4, space="PSUM") as psp,
        ):
            # ---- input DMAs (x first - encoder needs it asap) ----
            xt = enc.tile([D, 4 * XCOLS], BF16)
            nc.scalar.dma_start(xt[:, 0 : 2 * XCOLS], xT_d[:, 0 : 2 * XCOLS])
            nc.scalar.dma_start(
                xt[:, 2 * XCOLS : 4 * XCOLS], xT_d[:, 2 * XCOLS : 4 * XCOLS]
            )
            cpack = cst.tile([D, 6], F32)
            nc.sync.dma_start(cpack[:], cpack_d[:])
            wab = cst.tile([D, 6 * H], BF16)
            nc.sync.dma_start(wab[:], wab_d[:])
            w1bT = cst.tile([H, D], BF16)
            nc.sync.dma_start(w1bT[:, :], w1bT_d[:])
            p1 = cst.tile([D, 32 * H], BF16)
            nc.sync.dma_start(p1[:], p1_d[:])
            b1b = cpack[:, 1:2]
            b2a2 = cpack[:, 2:3]
            bcmb = cpack[0:H, 3:4]
            kcol = cpack[0:H, 4:5]
            bcmbl = cpack[0:H, 0:1]   # lam * bcmb
            b1bl = cpack[:, 5:6]      # lam * b1b
            wcmb = wab[:, 0 : 4 * H]       # W1a@W_enc, 4 k-tiles of [128,64]
            w2aT = wab[:, 4 * H : 6 * H]
            NCH = XCOLS // ECW  # 3 chunks of 352

            # ---- selu helper: v = psum+b (bf16); negative branch via
            # min(lam*al*(e^v - 1), 0) so exp needs no pre-clamp; final
            # clamp+add fused into one scalar_tensor_tensor ----
            # ACT-direct variant (both branches from PSUM) - used for the
            # a1 layer, where ACT still has headroom early in the prologue
            def selu_act(out_ap, pa, b_raw, b_lam, p, nm):
                r = pre.tile([p, ECW], BF16, tag="selr", name=f"r_{nm}")
                nc.scalar.activation(r[:], pa, AF.Relu, bias=b_lam, scale=LAM)
                e = pre.tile([p, ECW], BF16, tag="sele", name=f"e_{nm}")
                nc.scalar.activation(e[:], pa, AF.Exp, bias=b_raw)
                t = pre.tile([p, ECW], BF16, tag="selt", name=f"t_{nm}")
                nc.vector.tensor_scalar(t[:], e[:], LAM * AL, -LAM * AL, OP.mult, OP.add)
                nc.vector.scalar_tensor_tensor(
                    out_ap, t[:], 0.0, r[:], OP.min, OP.add
                )

            def selu_from_psum(out_ap, pa, b_raw, b_lam, p, nm):
                v = pre.tile([p, ECW], BF16, tag="selv", name=f"v_{nm}")
                nc.vector.tensor_scalar(v[:], pa, b_raw, 0.0, OP.add, OP.add)
                r = pre.tile([p, ECW], BF16, tag="selr", name=f"r_{nm}")
                nc.vector.tensor_scalar(r[:], v[:], 0.0, LAM, OP.max, OP.mult)
                e = pre.tile([p, ECW], BF16, tag="sele", name=f"e_{nm}")
                nc.scalar.activation(e[:], v[:], AF.Exp)
                t = pre.tile([p, ECW], BF16, tag="selt", name=f"t_{nm}")
                nc.vector.tensor_scalar(t[:], e[:], LAM * AL, -LAM * AL, OP.mult, OP.add)
                nc.vector.scalar_tensor_tensor(
                    out_ap, t[:], 0.0, r[:], OP.min, OP.add
                )

            # ---- wavefront emission over (layer, chunk): keeps every
            # engine queue supplied while chains pipeline ----
            a1T = enc.tile([H, XCOLS], BF16)
            hT = enc.tile([D, XCOLS], BF16)
            g2dup = cst.tile([D, XCOLS], F32)
            g2w = cst.tile([D, GSPAN], BF16)
            E2w = cst.tile([D, GSPAN], BF16)
            gbs_raw = cst.tile([D, PAIRS], F32)
            gbs_s = cst.tile([D, PAIRS], F32)
            EIS = cst.tile([D, PAIRS], F32)

            def do_a1(c):
                # fused encoder+first layer: a1pre = x @ (W1a@W_enc).T
                sl = slice(c * ECW, (c + 1) * ECW)
                pa = psp.tile([H, ECW], F32, tag="pp", name=f"pa_{c}")
                for k in range(4):
                    nc.tensor.matmul(
                        pa[:],
                        wcmb[:, k * H : (k + 1) * H],
                        xt[:, k * XCOLS + c * ECW : k * XCOLS + (c + 1) * ECW],
                        start=(k == 0),
                        stop=(k == 3),
                    )
                selu_act(a1T[:, sl], pa[:], bcmb, bcmbl, H, f"a{c}")

            def do_h(c):
                sl = slice(c * ECW, (c + 1) * ECW)
                ph = psp.tile([D, ECW], F32, tag="pp", name=f"ph_{c}")
                nc.tensor.matmul(ph[:], w1bT[:, :], a1T[:, sl], start=True, stop=True)
                selu_from_psum(hT[:, sl], ph[:], b1b, b1bl, D, f"h{c}")

            def do_g(c):
                sl = slice(c * ECW, (c + 1) * ECW)
                pg = psp.tile([D, ECW], F32, tag="pp", name=f"pg_{c}")
                nc.tensor.matmul(pg[:], w2aT, hT[:, sl], start=True, stop=True)
                nc.scalar.activation(g2dup[:, sl], pg[:], AF.Copy)
                # build the g2w/E2w pieces this chunk enables (top: same cols,
                # bottom: cols shifted by -96), so the main loop can start
                # right after the last chunk
                t0, t1 = c * ECW, min((c + 1) * ECW, GSPAN)
                if t1 > t0:
                    nc.gpsimd.tensor_scalar_mul(
                        g2w[0:H, t0:t1], g2dup[0:H, t0:t1], 1.0 / AL
                    )
                    nc.scalar.activation(
                        E2w[0:H, t0:t1], g2dup[0:H, t0:t1], AF.Exp,
                        bias=b2a2[0:H, :],
                    )
                b0 = max(c * ECW - 96, 0)
                b1 = min((c + 1) * ECW - 96, GSPAN)
                if b1 > b0:
                    nc.gpsimd.tensor_scalar_mul(
                        g2w[H:D, b0:b1], g2dup[H:D, 96 + b0 : 96 + b1], 1.0 / AL
                    )
                    nc.scalar.activation(
                        E2w[H:D, b0:b1], g2dup[H:D, 96 + b0 : 96 + b1], AF.Exp,
                        bias=b2a2[H:D, :],
                    )
                if c == 0:
                    nc.gpsimd.tensor_copy(gbs_raw[0:H, :], g2dup[0:H, 0:PAIRS])
                    nc.gpsimd.tensor_copy(
                        gbs_raw[H:D, :], g2dup[H:D, 96 : 96 + PAIRS]
                    )
                    nc.vector.tensor_scalar(
                        gbs_s[:], gbs_raw[:], b2a2, 1.0 / AL, OP.add, OP.mult
                    )
                    nc.scalar.activation(EIS[:], gbs_raw[:], AF.Exp)

            layers = (do_a1, do_h, do_g)
            for step in range(len(layers) + NCH - 1):
                for c in range(NCH):
                    li = step - c
                    if 0 <= li < len(layers):
                        layers[li](c)

            # ================= main pairwise loop =================
            def finalize(G, pb, gsz, pt, c0, cw):
                # o = Relu(psum/6 + kb) ; out = min(o, 1) ; DMA out.
                # pair s of the group lands on psum rows s (top) and 32+s
                # (bottom). The min runs on Pool mid-stream but on DVE for
                # the last group (DVE is idle by then; Pool's slower op sits
                # on the kernel's critical tail).
                o = opool.tile([H, BW], BF16, tag="o", name=f"o_{G}_{c0}")
                nc.scalar.activation(
                    o[:, c0 : c0 + cw], pt[:], AF.Relu,
                    scale=1.0 / 6.0, bias=kcol,
                )
                oo = opool.tile([H, BW], BF16, tag="oo", name=f"oo_{G}_{c0}")
                if G == len(GROUP_SIZES) - 1:
                    nc.vector.tensor_scalar_min(
                        oo[:, c0 : c0 + cw], o[:, c0 : c0 + cw], 1.0
                    )
                else:
                    nc.gpsimd.tensor_scalar_min(
                        oo[:, c0 : c0 + cw], o[:, c0 : c0 + cw], 1.0
                    )
                if gsz == 32:
                    # both row halves are contiguous: one descriptor
                    nc.sync.dma_start(
                        out_d[2 * pb : 2 * pb + 64, c0 : c0 + cw],
                        oo[:, c0 : c0 + cw],
                    )
                else:
                    nc.sync.dma_start(
                        out_d[2 * pb : 2 * pb + gsz, c0 : c0 + cw],
                        oo[0:gsz, c0 : c0 + cw],
                    )
                    nc.sync.dma_start(
                        out_d[2 * pb + gsz : 2 * pb + 2 * gsz, c0 : c0 + cw],
                        oo[32 : 32 + gsz, c0 : c0 + cw],
                    )

            pending_fin = []
            pbase = 0

            for G, GSZ in enumerate(GROUP_SIZES):
                pc0 = ps.tile([H, CW0], F32, tag="pc0", name=f"pc0_{G}")
                pc1 = ps.tile([H, CW1], F32, tag="pcx", name=f"pc1_{G}")
                started0 = False
                started1 = [False]

                # proportionally interleaved scheme schedule (Bresenham)
                if GROUP_QUOTAS is not None:
                    NZg, NYg, NWg, NWPg = GROUP_QUOTAS[G]
                else:
                    NZg, NYg, NWg, NWPg = NZ, NY, NW, NWP
                quota = {"Z": NZg * GSZ / 32.0, "Y2": NYg * GSZ / 32.0,
                         "W2": NWg * GSZ / 32.0}
                counts = {k: 0 for k in quota}
                sched = []
                for s in range(GSZ):
                    k = max(quota, key=lambda k: quota[k] * (s + 1) / GSZ - counts[k])
                    sched.append(k)
                    counts[k] += 1
                NWG = counts["W2"]
                NWPG = min(NWPg, NWG)
                w2_idx = 0
                # last slot must not be W2 (deferred W2 work flushes before it
                # so the group's stop-flag matmul stays last)
                if sched[GSZ - 1] == "W2":
                    for si in range(GSZ - 2, -1, -1):
                        if sched[si] != "W2":
                            sched[GSZ - 1], sched[si] = sched[si], sched[GSZ - 1]
                            break
                deferred = []

                for s in range(GSZ):
                    p = pbase + s
                    scheme = sched[s]
                    # q = min(Ei * E2j, 1) = e^{min(u,0)}   (always DVE, 4x)
                    q = fq.tile([D, BW], BF16, tag="q", name=f"q_{G}_{s}")
                    nc.vector.tensor_scalar(
                        q[:], E2w[:, p : p + BW], EIS[:, p : p + 1],
                        1.0, OP.mult, OP.min,
                    )
                    # rs = relu(u)/al : DVE for Z slots, ACT otherwise
                    rs = fq.tile([D, BW], BF16, tag="rs", name=f"rs_{G}_{s}")
                    if scheme == "Z":
                        nc.vector.tensor_scalar(
                            rs[:], g2w[:, p : p + BW], gbs_s[:, p : p + 1],
                            0.0, OP.add, OP.max,
                        )
                    else:
                        nc.scalar.activation(
                            rs[:], g2w[:, p : p + BW], AF.Relu,
                            bias=gbs_s[:, p : p + 1],
                        )
                    def emit_mms(slot, movers, last):
                        nonlocal started0
                        pat1 = p1[:, H * slot : H * (slot + 1)]
                        for pt, c0, cw in ((pc0, 0, CW0), (pc1, CW0, CW1)):
                            for mi, mv in enumerate(movers):
                                nc.tensor.matmul(
                                    pt[:], pat1, mv[:, c0 : c0 + cw],
                                    start=(not started0 and c0 == 0 and mi == 0)
                                    or (not started1[0] and c0 == CW0 and mi == 0),
                                    stop=last and mi == len(movers) - 1,
                                    skip_group_check=True,
                                )
                                if c0 == CW0:
                                    started1[0] = True
                        started0 = True

                    if s == GSZ - 1:
                        # flush deferred W2 work so this slot's mms close psum
                        for emit in deferred:
                            emit()
                        deferred = []

                    if scheme == "W2":
                        f = fq.tile([D, BW], BF16, tag="f", name=f"f_{G}_{s}")
                        on_pool = NWPG > 0 and (
                            (w2_idx + 1) * NWPG // NWG > w2_idx * NWPG // NWG
                        )
                        w2_idx += 1

                        def emit_w2(f=f, q=q, rs=rs, slot=s, on_pool=on_pool):
                            if on_pool:
                                nc.gpsimd.tensor_tensor(f[:], q[:], rs[:], OP.add)
                            else:
                                nc.vector.tensor_tensor(f[:], q[:], rs[:], OP.add)
                            emit_mms(slot, (f,), False)

                        deferred.append(emit_w2)
                    else:
                        emit_mms(s, (q, rs), s == GSZ - 1)
                        # drain one deferred W2 behind this slot's ready work
                        if len(deferred) > 1:
                            deferred.pop(0)()
                    # emit previous group's finalize mid-stream so it doesn't
                    # head-of-line-block this group's producer ops
                    if s == 6 and pending_fin:
                        for fin in pending_fin:
                            finalize(*fin)
                        pending_fin = []
                pending_fin.append((G, pbase, GSZ, pc0, 0, CW0))
                pending_fin.append((G, pbase, GSZ, pc1, CW0, CW1))
                pbase += GSZ
            for fin in pending_fin:
                finalize(*fin)

    nc.compile()
    return nc


def _host_inputs(x, W_enc, b_enc, W1a, b1a, W1b, b1b, W2a, b2a, W2b, b2b):
    w = W2b[0].astype(np.float64)
    SW = float(w.sum())
    KB = (-LAM * AL * SW + float(b2b[0])) / 6.0 + 0.5

    import ml_dtypes

    lam_al_w = (LAM * AL * w).astype(np.float32)
    p1 = np.zeros((D, 32 * H), np.float32)
    for s in range(32):
        p1[0:H, H * s + s] = lam_al_w
        p1[H:D, H * s + 32 + s] = lam_al_w

    W_combo = (W1a.astype(np.float64) @ W_enc.astype(np.float64)).astype(
        np.float32
    )  # [H, RAW]
    b_combo = (W1a.astype(np.float64) @ b_enc.astype(np.float64)
               + b1a.astype(np.float64)).astype(np.float32)
    cpack = np.zeros((D, 6), np.float32)
    cpack[0:H, 0] = LAM * b_combo
    cpack[:, 1] = b1b
    cpack[:, 2] = np.concatenate([b2a, b2a])
    cpack[0:H, 3] = b_combo
    cpack[0:H, 4] = KB
    cpack[:, 5] = LAM * b1b
    w2aT = np.ascontiguousarray(W2a.T)
    wcmbT = np.concatenate(
        [W_combo.T[k * D : (k + 1) * D] for k in range(4)], axis=1
    )  # [128, 4*64]
    wab = np.concatenate([wcmbT, w2aT, w2aT], axis=1)
    common = {
        "cpack": cpack,
        "wab": wab.astype(ml_dtypes.bfloat16),
        "w1bT": np.ascontiguousarray(W1b.T).astype(ml_dtypes.bfloat16),
        "p1": p1.astype(ml_dtypes.bfloat16),
    }
    in_maps = []
    for c in range(NCORES):
        m = dict(common)
        xr = np.roll(x, -ROWS * c, axis=0)[:XCOLS]
        xrT = np.ascontiguousarray(xr.T)  # [512, 1056]
        m["xT"] = np.concatenate(
            [xrT[k * D : (k + 1) * D] for k in range(4)], axis=1
        ).astype(ml_dtypes.bfloat16)
        in_maps.append(m)
    return in_maps


def _assemble(results):
    N = N_NODES
    O = np.zeros((N, N), np.float32)
    dd = np.arange(769)
    for c in range(NCORES):
        T = np.asarray(results[c]["out"][:, :769], np.float32)
        # rows grouped as (group, half, s) per GROUP_SIZES
        i_loc = np.empty(ROWS, np.int64)
        rbase = 0
        pbase = 0
        for gsz in GROUP_SIZES:
            sidx = np.arange(gsz)
            i_loc[rbase : rbase + gsz] = pbase + sidx
            i_loc[rbase + gsz : rbase + 2 * gsz] = pbase + sidx + 96
            rbase += 2 * gsz
            pbase += gsz
        gi = (ROWS * c + i_loc) % N
        cols = (gi[:, None] + dd[None, :]) % N
        O[gi[:, None], cols] = T
        O[cols, gi[:, None]] = T
    return O


def kernel(x, W_enc, b_enc, W1a, b1a, W1b, b1b, W2a, b2a, W2b, b2b):
    from concourse.bass_utils import run_bass_kernel_spmd

    global _compiled
    if _compiled is None:
        _compiled = _build_program()
    in_maps = _host_inputs(
        np.asarray(x, np.float32),
        np.asarray(W_enc, np.float32), np.asarray(b_enc, np.float32),
        np.asarray(W1a, np.float32), np.asarray(b1a, np.float32),
        np.asarray(W1b, np.float32), np.asarray(b1b, np.float32),
        np.asarray(W2a, np.float32), np.asarray(b2a, np.float32),
        np.asarray(W2b, np.float32), np.asarray(b2b, np.float32),
    )
    res = run_bass_kernel_spmd(_compiled, in_maps, list(range(NCORES)))
    return _assemble(res.results)


# revision 13
# speedup vs baseline: 1.0619x; 1.0020x over previous
"""Trainium2 Bass kernel v2 for nn_LCAMatrixModel (pairwise selu-MLP scoring).

o[i,j] = hardsigmoid( sum_h W2b[h]*selu(g[i,h]+g[j,h]+b2a[h]) + b2b ), symmetric.

Circulant band decomposition: each core owns 192 consecutive global rows
(core c: rows [192c, 192c+192)) of the band t[i, d] = o[i, (i+d) % N],
d in [0, 769) - exactly half the matrix. Inputs are host-rolled by 192c so
all cores run the same program on local rows [0, 192). Rows are processed
as 96 pairs (i, i+96) stacked on 128 partitions (2 x 64 h).

Per-pair math over the band window (u = g_i + g_j + b2a):
  sum_h w*selu(u) = lam*al*sum_h w*(rs + q) - lam*al*Sw,  with
  rs = relu(u)/al   and   q = e^{min(u,0)} = min(e^{g_i} * e^{g_j+b2a}, 1)
(the exp is SEPARABLE: E=e^g per node is precomputed, so the pairwise q is
just a multiply+min). o = clip(V/6 + 0.5, 0, 1). Every matmul uses the one
stationary pattern P1 = lam*al*w (g is stored pre-divided by al).

Engine balance via per-pair schemes (slot counts tunable):
  Z  : rs = DVE ts(add,max0);  q = DVE ts(mult,min1);  PE: P1@q + P1@rs
  Y2 : rs = ACT Relu(g2w+bias); q = DVE;               PE: P1@q + P1@rs
  W2 : rs = ACT; q = DVE; f = q+rs (DVE or Pool tt);   PE: P1@f  (half PE)
"""
import sys

sys.path.insert(0, "/opt/trn_rl_repo")

import numpy as np

N_NODES = 1536
RAW = 512
D = 128
H = 64
NCORES = 8
ROWS = N_NODES // NCORES          # 192 rows per core
PAIRS = ROWS // 2                 # 96 pairs (i, i+96)
GROUPS = 3                        # 3 groups of 32 pairs
BW = 769                          # band width
GSPAN = 868                       # g2w width: max i (95) + BW + 1
XCOLS = 966                       # g cols needed: >= 963 (bottom shift+span)
CW0, CW1 = 356, BW - 356          # psum chunk widths (356+413)
ECW = 322                         # encoder/prologue chunk width (3 x 322 = 966)

LAM = 1.0507009873554805
AL = 1.6732632423543772

# per-group scheme quotas (fractions of 32): Z=DVE rs, Y2=ACT rs + 2mm,
# W2=ACT rs + f-combine + 1mm. Of W2, NWP_FR on Pool, rest DVE.
NZ_DEF = 15
NY_DEF = 3
NWP_DEF = 7
GROUP_SIZES = (32, 32, 32)
# per-group (NZ, NY, NW, NWP): front-load PE work, drain PE-light
GROUP_QUOTAS = None  # uniform per-group quotas

_compiled = None


def _build_program(NZ=NZ_DEF, NY=NY_DEF, NWP=NWP_DEF):
    import concourse.bacc as bacc
    import concourse.mybir as mybir
    import concourse.tile as tile

    F32 = mybir.dt.float32
    BF16 = mybir.dt.bfloat16
    AF = mybir.ActivationFunctionType
    OP = mybir.AluOpType

    NW = 32 - NZ - NY

    nc = bacc.Bacc("TRN2", target_bir_lowering=False, debug=False)

    # ---- DRAM I/O ----
    xT_d = nc.dram_tensor("xT", [D, 4 * XCOLS], BF16, kind="ExternalInput")
    cpack_d = nc.dram_tensor("cpack", [D, 6], F32, kind="ExternalInput")
    wab_d = nc.dram_tensor("wab", [D, 6 * H], BF16, kind="ExternalInput")
    w1bT_d = nc.dram_tensor("w1bT", [H, D], BF16, kind="ExternalInput")
    p1_d = nc.dram_tensor("p1", [D, 32 * H], BF16, kind="ExternalInput")
    out_d = nc.dram_tensor("out", [ROWS, BW], BF16, kind="ExternalOutput")

    with tile.TileContext(nc) as tc:
        with (
            tc.tile_pool(name="cst", bufs=1) as cst,
            tc.tile_pool(name="enc", bufs=1) as enc,
            tc.tile_pool(name="pre", bufs=3) as pre,
            tc.tile_pool(name="fq", bufs=28) as fq,
            tc.tile_pool(name="op", bufs=6) as opool,
            tc.tile_pool(name="ps", bufs=2, space="PSUM") as ps,
            tc.tile_pool(name="psp", bufs=4, space="PSUM") as psp,
        ):
            # ---- input DMAs (x first - encoder needs it asap) ----
            xt = enc.tile([D, 4 * XCOLS], BF16)
            nc.scalar.dma_start(
                xt[:, 0 : 3 * XCOLS], xT_d[:, 0 : 3 * XCOLS]
            )
            nc.scalar.dma_start(
                xt[:, 3 * XCOLS : 4 * XCOLS], xT_d[:, 3 * XCOLS : 4 * XCOLS]
            )
            cpack = cst.tile([D, 6], F32)
            nc.sync.dma_start(cpack[:], cpack_d[:])
            wab = cst.tile([D, 6 * H], BF16)
            nc.sync.dma_start(wab[:], wab_d[:])
            w1bT = cst.tile([H, D], BF16)
            nc.sync.dma_start(w1bT[:, :], w1bT_d[:])
            p1 = cst.tile([D, 32 * H], BF16)
            nc.sync.dma_start(p1[:], p1_d[:])
            b1b = cpack[:, 1:2]
            b2a2 = cpack[:, 2:3]
            bcmb = cpack[0:H, 3:4]
            kcol = cpack[0:H, 4:5]
            bcmbl = cpack[0:H, 0:1]   # lam * bcmb
            b1bl = cpack[:, 5:6]      # lam * b1b
            wcmb = wab[:, 0 : 4 * H]       # W1a@W_enc, 4 k-tiles of [128,64]
            w2aT = wab[:, 4 * H : 6 * H]
            NCH = XCOLS // ECW  # 3 chunks of 352

            # ---- selu helper: v = psum+b (bf16); negative branch via
            # min(lam*al*(e^v - 1), 0) so exp needs no pre-clamp; final
            # clamp+add fused into one scalar_tensor_tensor ----
            # ACT-direct variant (both branches from PSUM) - used for the
            # a1 layer, where ACT still has headroom early in the prologue
            def selu_act(out_ap, pa, b_raw, b_lam, p, nm):
                r = pre.tile([p, ECW], BF16, tag="selr", name=f"r_{nm}")
                nc.scalar.activation(r[:], pa, AF.Relu, bias=b_lam, scale=LAM)
                e = pre.tile([p, ECW], BF16, tag="sele", name=f"e_{nm}")
                nc.scalar.activation(e[:], pa, AF.Exp, bias=b_raw)
                t = pre.tile([p, ECW], BF16, tag="selt", name=f"t_{nm}")
                nc.vector.tensor_scalar(t[:], e[:], LAM * AL, -LAM * AL, OP.mult, OP.add)
                nc.vector.scalar_tensor_tensor(
                    out_ap, t[:], 0.0, r[:], OP.min, OP.add
                )

            def selu_from_psum(out_ap, pa, b_raw, b_lam, p, nm):
                v = pre.tile([p, ECW], BF16, tag="selv", name=f"v_{nm}")
                nc.vector.tensor_scalar(v[:], pa, b_raw, 0.0, OP.add, OP.add)
                r = pre.tile([p, ECW], BF16, tag="selr", name=f"r_{nm}")
                nc.vector.tensor_scalar(r[:], v[:], 0.0, LAM, OP.max, OP.mult)
                e = pre.tile([p, ECW], BF16, tag="sele", name=f"e_{nm}")
                nc.scalar.activation(e[:], v[:], AF.Exp)
                t = pre.tile([p, ECW], BF16, tag="selt", name=f"t_{nm}")
                nc.vector.tensor_scalar(t[:], e[:], LAM * AL, -LAM * AL, OP.mult, OP.add)
                nc.vector.scalar_tensor_tensor(
                    out_ap, t[:], 0.0, r[:], OP.min, OP.add
                )

            # ---- wavefront emission over (layer, chunk): keeps every
            # engine queue supplied while chains pipeline ----
            a1T = enc.tile([H, XCOLS], BF16)
            hT = enc.tile([D, XCOLS], BF16)
            g2dup = cst.tile([D, XCOLS], F32)
            g2w = cst.tile([D, GSPAN], BF16)
            E2w = cst.tile([D, GSPAN], BF16)
            gbs_raw = cst.tile([D, PAIRS], F32)
            gbs_s = cst.tile([D, PAIRS], F32)
            EIS = cst.tile([D, PAIRS], F32)

            def do_a1(c):
                # fused encoder+first layer: a1pre = x @ (W1a@W_enc).T
                sl = slice(c * ECW, (c + 1) * ECW)
                pa = psp.tile([H, ECW], F32, tag="pp", name=f"pa_{c}")
                for k in range(4):
                    nc.tensor.matmul(
                        pa[:],
                        wcmb[:, k * H : (k + 1) * H],
                        xt[:, k * XCOLS + c * ECW : k * XCOLS + (c + 1) * ECW],
                        start=(k == 0),
                        stop=(k == 3),
                    )
                selu_act(a1T[:, sl], pa[:], bcmb, bcmbl, H, f"a{c}")

            def do_h(c):
                sl = slice(c * ECW, (c + 1) * ECW)
                ph = psp.tile([D, ECW], F32, tag="pp", name=f"ph_{c}")
                nc.tensor.matmul(ph[:], w1bT[:, :], a1T[:, sl], start=True, stop=True)
                selu_from_psum(hT[:, sl], ph[:], b1b, b1bl, D, f"h{c}")

            def do_g(c):
                sl = slice(c * ECW, (c + 1) * ECW)
                pg = psp.tile([D, ECW], F32, tag="pp", name=f"pg_{c}")
                nc.tensor.matmul(pg[:], w2aT, hT[:, sl], start=True, stop=True)
                nc.scalar.activation(g2dup[:, sl], pg[:], AF.Copy)
                # build the g2w/E2w pieces this chunk enables (top: same cols,
                # bottom: cols shifted by -96), so the main loop can start
                # right after the last chunk
                t0, t1 = c * ECW, min((c + 1) * ECW, GSPAN)
                if t1 > t0:
                    nc.gpsimd.tensor_scalar_mul(
                        g2w[0:H, t0:t1], g2dup[0:H, t0:t1], 1.0 / AL
                    )
                    nc.scalar.activation(
                        E2w[0:H, t0:t1], g2dup[0:H, t0:t1], AF.Exp,
                        bias=b2a2[0:H, :],
                    )
                b0 = max(c * ECW - 96, 0)
                b1 = min((c + 1) * ECW - 96, GSPAN)
                if b1 > b0:
                    nc.gpsimd.tensor_scalar_mul(
                        g2w[H:D, b0:b1], g2dup[H:D, 96 + b0 : 96 + b1], 1.0 / AL
                    )
                    nc.scalar.activation(
                        E2w[H:D, b0:b1], g2dup[H:D, 96 + b0 : 96 + b1], AF.Exp,
                        bias=b2a2[H:D, :],
                    )
                if c == 0:
                    nc.gpsimd.tensor_copy(gbs_raw[0:H, :], g2dup[0:H, 0:PAIRS])
                    nc.gpsimd.tensor_copy(
                        gbs_raw[H:D, :], g2dup[H:D, 96 : 96 + PAIRS]
                    )
                    nc.vector.tensor_scalar(
                        gbs_s[:], gbs_raw[:], b2a2, 1.0 / AL, OP.add, OP.mult
                    )
                    nc.scalar.activation(EIS[:], gbs_raw[:], AF.Exp)

            layers = (do_a1, do_h, do_g)
            for step in range(len(layers) + NCH - 1):
                for c in range(NCH):
                    li = step - c
                    if 0 <= li < len(layers):
                        layers[li](c)

            # ================= main pairwise loop =================
            def finalize(G, pb, gsz, pt, c0, cw):
                # o = Relu(psum/6 + kb) ; out = min(o, 1) ; DMA out.
                # pair s of the group lands on psum rows s (top) and 32+s
                # (bottom). The min runs on Pool mid-stream but on DVE for
                # the last group (DVE is idle by then; Pool's slower op sits
                # on the kernel's critical tail).
                o = opool.tile([H, BW], BF16, tag="o", name=f"o_{G}_{c0}")
                nc.scalar.activation(
                    o[:, c0 : c0 + cw], pt[:], AF.Relu,
                    scale=1.0 / 6.0, bias=kcol,
                )
                oo = opool.tile([H, BW], BF16, tag="oo", name=f"oo_{G}_{c0}")
                if G == len(GROUP_SIZES) - 1:
                    nc.vector.tensor_scalar_min(
                        oo[:, c0 : c0 + cw], o[:, c0 : c0 + cw], 1.0
                    )
                else:
                    nc.gpsimd.tensor_scalar_min(
                        oo[:, c0 : c0 + cw], o[:, c0 : c0 + cw], 1.0
                    )
                if gsz == 32:
                    # both row halves are contiguous: one descriptor
                    nc.sync.dma_start(
                        out_d[2 * pb : 2 * pb + 64, c0 : c0 + cw],
                        oo[:, c0 : c0 + cw],
                    )
                else:
                    nc.sync.dma_start(
                        out_d[2 * pb : 2 * pb + gsz, c0 : c0 + cw],
                        oo[0:gsz, c0 : c0 + cw],
                    )
                    nc.sync.dma_start(
                        out_d[2 * pb + gsz : 2 * pb + 2 * gsz, c0 : c0 + cw],
                        oo[32 : 32 + gsz, c0 : c0 + cw],
                    )

            pending_fin = []
            pbase = 0

            for G, GSZ in enumerate(GROUP_SIZES):
                pc0 = ps.tile([H, CW0], F32, tag="pc0", name=f"pc0_{G}")
                pc1 = ps.tile([H, CW1], F32, tag="pcx", name=f"pc1_{G}")
                started0 = False
                started1 = [False]

                # proportionally interleaved scheme schedule (Bresenham)
                if GROUP_QUOTAS is not None:
                    NZg, NYg, NWg, NWPg = GROUP_QUOTAS[G]
                else:
                    NZg, NYg, NWg, NWPg = NZ, NY, NW, NWP
                quota = {"Z": NZg * GSZ / 32.0, "Y2": NYg * GSZ / 32.0,
                         "W2": NWg * GSZ / 32.0}
                counts = {k: 0 for k in quota}
                sched = []
                for s in range(GSZ):
                    k = max(quota, key=lambda k: quota[k] * (s + 1) / GSZ - counts[k])
                    sched.append(k)
                    counts[k] += 1
                NWG = counts["W2"]
                NWPG = min(NWPg, NWG)
                w2_idx = 0
                # last slot must not be W2 (deferred W2 work flushes before it
                # so the group's stop-flag matmul stays last)
                if sched[GSZ - 1] == "W2":
                    for si in range(GSZ - 2, -1, -1):
                        if sched[si] != "W2":
                            sched[GSZ - 1], sched[si] = sched[si], sched[GSZ - 1]
                            break
                deferred = []

                for s in range(GSZ):
                    p = pbase + s
                    scheme = sched[s]
                    # q = min(Ei * E2j, 1) = e^{min(u,0)}   (always DVE, 4x)
                    q = fq.tile([D, BW], BF16, tag="q", name=f"q_{G}_{s}")
                    nc.vector.tensor_scalar(
                        q[:], E2w[:, p : p + BW], EIS[:, p : p + 1],
                        1.0, OP.mult, OP.min,
                    )
                    # rs = relu(u)/al : DVE for Z slots, ACT otherwise
                    rs = fq.tile([D, BW], BF16, tag="rs", name=f"rs_{G}_{s}")
                    if scheme == "Z":
                        nc.vector.tensor_scalar(
                            rs[:], g2w[:, p : p + BW], gbs_s[:, p : p + 1],
                            0.0, OP.add, OP.max,
                        )
                    else:
                        nc.scalar.activation(
                            rs[:], g2w[:, p : p + BW], AF.Relu,
                            bias=gbs_s[:, p : p + 1],
                        )
                    def emit_mms(slot, movers, last):
                        nonlocal started0
                        pat1 = p1[:, H * slot : H * (slot + 1)]
                        for pt, c0, cw in ((pc0, 0, CW0), (pc1, CW0, CW1)):
                            for mi, mv in enumerate(movers):
                                nc.tensor.matmul(
                                    pt[:], pat1, mv[:, c0 : c0 + cw],
                                    start=(not started0 and c0 == 0 and mi == 0)
                                    or (not started1[0] and c0 == CW0 and mi == 0),
                                    stop=last and mi == len(movers) - 1,
                                    skip_group_check=True,
                                )
                                if c0 == CW0:
                                    started1[0] = True
                        started0 = True

                    if s == GSZ - 1:
                        # flush deferred W2 work so this slot's mms close psum
                        for emit in deferred:
                            emit()
                        deferred = []

                    if scheme == "W2":
                        f = fq.tile([D, BW], BF16, tag="f", name=f"f_{G}_{s}")
                        on_pool = NWPG > 0 and (
                            (w2_idx + 1) * NWPG // NWG > w2_idx * NWPG // NWG
                        )
                        w2_idx += 1

                        def emit_w2(f=f, q=q, rs=rs, slot=s, on_pool=on_pool):
                            if on_pool:
                                nc.gpsimd.tensor_tensor(f[:], q[:], rs[:], OP.add)
                            else:
                                nc.vector.tensor_tensor(f[:], q[:], rs[:], OP.add)
                            emit_mms(slot, (f,), False)

                        deferred.append(emit_w2)
                    else:
                        emit_mms(s, (q, rs), s == GSZ - 1)
                        # drain one deferred W2 behind this slot's ready work
                        if len(deferred) > 1:
                            deferred.pop(0)()
                    # emit previous group's finalize mid-stream so it doesn't
                    # head-of-line-block this group's producer ops
                    if s == 6 and pending_fin:
                        for fin in pending_fin:
                            finalize(*fin)
                        pending_fin = []
                pending_fin.append((G, pbase, GSZ, pc0, 0, CW0))
                pending_fin.append((G, pbase, GSZ, pc1, CW0, CW1))
                pbase += GSZ
            for fin in pending_fin:
                finalize(*fin)

    nc.compile()
    return nc


def _host_inputs(x, W_enc, b_enc, W1a, b1a, W1b, b1b, W2a, b2a, W2b, b2b):
    w = W2b[0].astype(np.float64)
    SW = float(w.sum())
    KB = (-LAM * AL * SW + float(b2b[0])) / 6.0 + 0.5

    import ml_dtypes

    lam_al_w = (LAM * AL * w).astype(np.float32)
    p1 = np.zeros((D, 32 * H), np.float32)
    for s in range(32):
        p1[0:H, H * s + s] = lam_al_w
        p1[H:D, H * s + 32 + s] = lam_al_w

    W_combo = (W1a.astype(np.float64) @ W_enc.astype(np.float64)).astype(
        np.float32
    )  # [H, RAW]
    b_combo = (W1a.astype(np.float64) @ b_enc.astype(np.float64)
               + b1a.astype(np.float64)).astype(np.float32)
    cpack = np.zeros((D, 6), np.float32)
    cpack[0:H, 0] = LAM * b_combo
    cpack[:, 1] = b1b
    cpack[:, 2] = np.concatenate([b2a, b2a])
    cpack[0:H, 3] = b_combo
    cpack[0:H, 4] = KB
    cpack[:, 5] = LAM * b1b
    w2aT = np.ascontiguousarray(W2a.T)
    wcmbT = np.concatenate(
        [W_combo.T[k * D : (k + 1) * D] for k in range(4)], axis=1
    )  # [128, 4*64]
    wab = np.concatenate([wcmbT, w2aT, w2aT], axis=1)
    common = {
        "cpack": cpack,
        "wab": wab.astype(ml_dtypes.bfloat16),
        "w1bT": np.ascontiguousarray(W1b.T).astype(ml_dtypes.bfloat16),
        "p1": p1.astype(ml_dtypes.bfloat16),
    }
    in_maps = []
    for c in range(NCORES):
        m = dict(common)
        xr = np.roll(x, -ROWS * c, axis=0)[:XCOLS]
        xrT = np.ascontiguousarray(xr.T)  # [512, 1056]
        m["xT"] = np.concatenate(
            [xrT[k * D : (k + 1) * D] for k in range(4)], axis=1
        ).astype(ml_dtypes.bfloat16)
        in_maps.append(m)
    return in_maps


def _assemble(results):
    N = N_NODES
    O = np.zeros((N, N), np.float32)
    dd = np.arange(769)
    for c in range(NCORES):
        T = np.asarray(results[c]["out"][:, :769], np.float32)
        # rows grouped as (group, half, s) per GROUP_SIZES
        i_loc = np.empty(ROWS, np.int64)
        rbase = 0
        pbase = 0
        for gsz in GROUP_SIZES:
            sidx = np.arange(gsz)
            i_loc[rbase : rbase + gsz] = pbase + sidx
            i_loc[rbase + gsz : rbase + 2 * gsz] = pbase + sidx + 96
            rbase += 2 * gsz
            pbase += gsz
        gi = (ROWS * c + i_loc) % N
        cols = (gi[:, None] + dd[None, :]) % N
        O[gi[:, None], cols] = T
        O[cols, gi[:, None]] = T
    return O


def kernel(x, W_enc, b_enc, W1a, b1a, W1b, b1b, W2a, b2a, W2b, b2b):
    from concourse.bass_utils import run_bass_kernel_spmd

    global _compiled
    if _compiled is None:
        _compiled = _build_program()
    in_maps = _host_inputs(
        np.asarray(x, np.float32),
        np.asarray(W_enc, np.float32), np.asarray(b_enc, np.float32),
        np.asarray(W1a, np.float32), np.asarray(b1a, np.float32),
        np.asarray(W1b, np.float32), np.asarray(b1b, np.float32),
        np.asarray(W2a, np.float32), np.asarray(b2a, np.float32),
        np.asarray(W2b, np.float32), np.asarray(b2b, np.float32),
    )
    res = run_bass_kernel_spmd(_compiled, in_maps, list(range(NCORES)))
    return _assemble(res.results)
